# revision 1
# baseline (speedup 1.0000x reference)
"""Distributed Trainium2 kernel for nn_Attention_64742337020012.

B=4, N=2048, E=768, H=12, D=64 causal attention with per-head RMS norm,
interleaved xpos RoPE, and output projection.

Sharding: 8 cores, core c owns batch c//2 and heads 6*(c%2) .. 6*(c%2)+6
(head-independent attention).  Each core computes full causal attention for
its 6 heads over all 2048 positions plus the partial output projection using
its 384 rows of proj_w^T; the host sums the two partial projections per batch
and adds the bias.

Device pipeline per core (all matmuls bf16, f32 accumulation):
  1. q,k loaded bf16 (cast-DMA), RMS-normed + roped via host-precomputed
     coefficient tables (rope pair-swap is expressed as two strided
     multiply-adds; the head dim is pre-permuted evens-first on the host so
     the swap reads are contiguous blocks).
  2. q',k' transposed to [d, n] layout via DMA xbar transpose (bf16).
  3. Scores computed transposed (S^T[k,q] tiles), exp on ACT straight out of
     PSUM (no max subtraction: RMS norm bounds |q.k| <= 64, and xpos decay
     keeps the in-tile non-causal overhang finite), causal masking by a 0/1
     multiply on the diagonal tiles only.
  4. AV with P^T stationary and [V | 1] moving gives y and the softmax
     denominator in one accumulation; per-row reciprocal normalizes.
  5. y transposed (DMA xbar) and projected against the core's slice of
     proj_w^T; f32 partial written out.
"""

import sys

sys.path.insert(0, "/opt/trn_rl_repo")

import numpy as np
import ml_dtypes

import concourse.bass as bass
import concourse.mybir as mybir
import concourse.tile as tile
from concourse.bass_utils import run_bass_kernel_spmd

# ----------------------------------------------------------------------------
# Workaround for this container's walrus build: the TileContext tail drain
# carries one SyncWait per outstanding semaphore, but CoreV3 CTRL codegen
# accepts only a single sync wait per instruction.  Split the waits across
# single-wait NOPs emitted right after the drain.
from concourse.vector_clock import ScopedClock as _ScopedClock


def _split_sync_waits(nc, inst, max_waits=1):
    si = inst.ins.sync_info
    if si is None:
        return
    waits = list(si.on_wait)
    if len(waits) <= max_waits:
        return
    inst.ins.sync_info = mybir.SyncInfo(
        on_wait=waits[:max_waits], on_update=list(si.on_update)
    )
    for i in range(max_waits, len(waits), max_waits):
        nop = nc.sync.nop(nofuse=True, hint="drain_wait_split")
        nop.ins.sync_info = mybir.SyncInfo(
            on_wait=waits[i : i + max_waits], on_update=[]
        )


def _patched_drain_and_barrier(self, tick_clock, wait_clock):
    nc = self.nc
    drain_inst = nc.sync.drain()
    wait_clock.add_sem_waits(
        drain_inst.ins, _ScopedClock({None: tick_clock.global_clock})
    )
    _split_sync_waits(nc, drain_inst)
    nc.all_engine_barrier()
    assert self.sems is not None
    popped = nc._tile_sem_poison_stack.pop()
    assert popped is self._sem_poison
    nc.clear_and_free_semaphores(list(self.sems.allocated().values()))
    nc.all_engine_barrier()


tile.TileContext._drain_and_barrier = _patched_drain_and_barrier


# Same walrus limitation, applied globally: any instruction carrying more
# than one SyncWait gets the extra waits hoisted onto same-engine NoOps
# inserted immediately before it in the BIR json (equivalent semantics: the
# engine's sequencer performs the waits in sequence).
import json as _json
import concourse.bass2jax as _bass2jax

_orig_compile_bir_kernel = _bass2jax.compile_bir_kernel


def _split_waits_in_bir(bir_json: bytes) -> bytes:
    j = _json.loads(bir_json)
    n_new = [0]
    for fn in j["functions"]:
        for bb in fn["blocks"]:
            insts = bb["instructions"]
            out = []
            for inst in insts:
                si = inst.get("sync_info")
                waits = (si or {}).get("on_wait") or []
                if len(waits) > 1:
                    for w in waits[:-1]:
                        n_new[0] += 1
                        out.append({
                            "engine": inst["engine"],
                            "ins": [], "outs": [],
                            "name": f"{inst['name']}-ws{n_new[0]}",
                            "opcode": "NoOp",
                            "sync_info": {"on_wait": [w], "on_update": []},
                        })
                    si["on_wait"] = [waits[-1]]
                out.append(inst)
            bb["instructions"] = out
    return _json.dumps(j).encode()


def _patched_compile_bir_kernel(bir_json, tmpdir, neff_name="file.neff"):
    return _orig_compile_bir_kernel(_split_waits_in_bir(bir_json), tmpdir, neff_name)


_bass2jax.compile_bir_kernel = _patched_compile_bir_kernel
# ----------------------------------------------------------------------------

B, N, E, H = 4, 2048, 768, 12
D = 64
RDIM = 32
EPS = 1e-6
XPOS_SCALE_BASE = 512.0
THETA = 10000.0

SC_SCALE = 0.125  # 1/sqrt(D)
HL = 6            # heads per core
EL = HL * D       # 384 local embed cols
P = 128
NT = N // P       # 16 row tiles
QC = 512          # q chunk (columns of S^T tiles)
NQC = N // QC     # 4
F32 = mybir.dt.float32
BF16 = mybir.dt.bfloat16

_CACHE = {}


def _head_perm():
    """Per-head column permutation: rotary evens, rotary odds, passthrough."""
    p = list(range(0, RDIM, 2)) + list(range(1, RDIM, 2)) + list(range(RDIM, D))
    return np.array(p, dtype=np.int64)


def _rope_tables(scale_vec):
    """cosPt [N, 64], sinPt [N, 32] coefficient tables in permuted layout.

    slot i (i<16):    out = x_e[i]*cosPt[i] + x_o[i]*sinPt[i]
    slot 16+i:        out = x_o[i]*cosPt[16+i] + x_e[i]*sinPt[16+i]
    slot 32+j:        out = x[32+j]*cosPt[32+j]
    scale_vec: rms scale (q_scale/k_scale), indexed in ORIGINAL layout.
    Returns (cosPt, sinPt) including the xpos scale (pass 1/xsc for k).
    """
    inv_freq = 1.0 / (THETA ** (np.arange(0, RDIM, 2, dtype=np.float64) / RDIM))
    t = np.arange(N, dtype=np.float64)
    freqs = t[:, None] * inv_freq[None, :]           # [N, 16]
    cos0, sin0 = np.cos(freqs), np.sin(freqs)
    base = (np.arange(0, RDIM, 2, dtype=np.float64) + 0.4 * RDIM) / (1.4 * RDIM)
    power = (t - N // 2) / XPOS_SCALE_BASE
    xsc = base[None, :] ** power[:, None]            # [N, 16]
    return cos0, sin0, xsc


def _build_tables(scale_vec, invert_xpos):
    cos0, sin0, xsc = _rope_tables(scale_vec)
    if invert_xpos:
        xsc = 1.0 / xsc
    sc = np.asarray(scale_vec, dtype=np.float64)
    cosPt = np.empty((N, D), dtype=np.float64)
    sinPt = np.empty((N, RDIM), dtype=np.float64)
    i = np.arange(16)
    cosPt[:, 0:16] = cos0 * xsc * sc[2 * i][None, :]
    cosPt[:, 16:32] = cos0 * xsc * sc[2 * i + 1][None, :]
    cosPt[:, 32:] = sc[RDIM:][None, :]
    sinPt[:, 0:16] = -sin0 * xsc * sc[2 * i + 1][None, :]
    sinPt[:, 16:32] = sin0 * xsc * sc[2 * i][None, :]
    return cosPt.astype(np.float32), sinPt.astype(np.float32)


def _build_masks():
    """mask[doff][kk, qq] = 1 if qq >= kk + 128*doff else 0; [4, 128, 512]."""
    kk = np.arange(P)[:, None]
    qq = np.arange(QC)[None, :]
    return np.stack(
        [(qq >= kk + P * doff).astype(np.float32) for doff in range(4)]
    )


def build_graph(reps=1):
    nc = bass.Bass()
    q_ext = nc.declare_dram_parameter("q", [N, EL], BF16, isOutput=False)
    k_ext = nc.declare_dram_parameter("k", [N, EL], BF16, isOutput=False)
    v_ext = nc.declare_dram_parameter("v", [P, NT * HL * (D + 1)], BF16, isOutput=False)
    wt_ext = nc.declare_dram_parameter("wt", [P, 3 * E], BF16, isOutput=False)
    qcos_ext = nc.declare_dram_parameter("qcos", [P, NT * D], BF16, isOutput=False)
    qsin_ext = nc.declare_dram_parameter("qsin", [P, NT * RDIM], BF16, isOutput=False)
    kcos_ext = nc.declare_dram_parameter("kcos", [P, NT * D], BF16, isOutput=False)
    ksin_ext = nc.declare_dram_parameter("ksin", [P, NT * RDIM], BF16, isOutput=False)
    mask_ext = nc.declare_dram_parameter("masks", [P, 4 * QC], BF16, isOutput=False)
    out_ext = nc.declare_dram_parameter("out", [N, E], F32, isOutput=True)

    q_t = q_ext.rearrange("(t p) e -> t p e", p=P)
    k_t = k_ext.rearrange("(t p) e -> t p e", p=P)
    v_t = v_ext
    out_t = out_ext.rearrange("(t p) e -> t p e", p=P)

    # fast-exp constants: exp(SC*s) ~ bitcast_f32(int32(ACON*s + BCON))
    ACON = float(SC_SCALE * (2.0 ** 23) / np.log(2.0))
    BCON = float(127 * (2 ** 23) - 486411)

    def bcast_heads(ap, nh=HL):
        return bass.AP(tensor=ap.tensor, offset=ap.offset,
                       ap=[ap.ap[0], [0, nh], ap.ap[1]])

    with tile.TileContext(nc) as tc:
        with (
            tc.tile_pool(name="persist", bufs=1) as persist,
            tc.tile_pool(name="qk_in", bufs=5) as qk_in,
            tc.tile_pool(name="pp", bufs=8) as pp,
            tc.tile_pool(name="pp_small", bufs=4) as pp_small,
            tc.tile_pool(name="pt_pool", bufs=5) as pt_pool,
            tc.tile_pool(name="fe_pool", bufs=3) as fe_pool,
            tc.tile_pool(name="ypre", bufs=4) as ypre_pool,
            tc.tile_pool(name="recip", bufs=8) as recip_pool,
            tc.tile_pool(name="outsb", bufs=4) as outsb_pool,
            tc.tile_pool(name="ps_s", bufs=2, space="PSUM") as ps_s,
            tc.tile_pool(name="ps_y", bufs=2, space="PSUM") as ps_y,
            tc.tile_pool(name="ps_o", bufs=1, space="PSUM") as ps_o,
        ):
            # ---------------- constants (host-prepared layouts) ----------------
            ident = persist.tile([P, P], BF16)
            from concourse.masks import make_identity
            make_identity(nc, ident)
            qcos_sb = persist.tile([P, NT, D], BF16)
            qsin_sb = persist.tile([P, NT, RDIM], BF16)
            kcos_sb = persist.tile([P, NT, D], BF16)
            ksin_sb = persist.tile([P, NT, RDIM], BF16)
            nc.sync.dma_start(out=qcos_sb[:], in_=qcos_ext[:])
            nc.sync.dma_start(out=qsin_sb[:], in_=qsin_ext[:])
            nc.sync.dma_start(out=kcos_sb[:], in_=kcos_ext[:])
            nc.sync.dma_start(out=ksin_sb[:], in_=ksin_ext[:])
            masks_sb = persist.tile([P, 4, QC], BF16)
            wt_sb = persist.tile([P, 3, E], BF16)
            vall = persist.tile([P, NT, HL, D + 1], BF16)

            qT = [[persist.tile([P, QC], BF16, name=f"qT{i}_{c}") for c in range(4)] for i in range(3)]
            kT = [[persist.tile([P, QC], BF16, name=f"kT{i}_{c}") for c in range(4)] for i in range(3)]
            yT = [[persist.tile([P, P], BF16, name=f"yT{i}_{t}") for t in range(NT)] for i in range(3)]

            # ---------------- preprocess q, k ----------------
            def preprocess_pair(i):
                    deng = nc.gpsimd if i < 4 else nc.sync
                    xk = qk_in.tile([P, HL, D], BF16, tag="xk_in", name="xk")
                    deng.dma_start(out=xk[:], in_=k_t[i])
                    xq = qk_in.tile([P, HL, D], BF16, tag="xq_in", name="xq")
                    deng.dma_start(out=xq[:], in_=q_t[i])
                    sqk = pp.tile([P, HL, D], BF16, tag="sqk", name="sqk")
                    nc.gpsimd.tensor_mul(sqk[:], xk[:], xk[:])
                    sqq = pp.tile([P, HL, D], BF16, tag="sqq", name="sqq")
                    nc.gpsimd.tensor_mul(sqq[:], xq[:], xq[:])
                    ssum = pp_small.tile([P, 2 * HL], F32, tag="ssum", name="ssum")
                    nc.vector.reduce_sum(ssum[:, 0:HL], sqk[:], axis=mybir.AxisListType.X)
                    nc.vector.reduce_sum(ssum[:, HL:], sqq[:], axis=mybir.AxisListType.X)
                    # rstd = rsqrt(ssum/64 + eps): Quake seed + 2 Newton steps,
                    # batched over k and q heads (DVE only, no ACT tables)
                    m = pp_small.tile([P, 2 * HL], F32, tag="m_ms", name="m")
                    nc.vector.tensor_scalar(out=m[:], in0=ssum[:], scalar1=1.0 / D,
                                            scalar2=EPS, op0=mybir.AluOpType.mult,
                                            op1=mybir.AluOpType.add)
                    ish = pp_small.tile([P, 2 * HL], mybir.dt.int32, tag="ish", name="ish")
                    nc.vector.tensor_scalar(out=ish[:], in0=m.bitcast(mybir.dt.int32),
                                            scalar1=1, scalar2=None,
                                            op0=mybir.AluOpType.logical_shift_right)
                    y0i = pp_small.tile([P, 2 * HL], mybir.dt.int32, tag="y0i", name="y0i")
                    nc.vector.tensor_scalar(out=y0i[:], in0=ish[:],
                                            scalar1=-1,
                                            scalar2=0x5F3759DF,
                                            op0=mybir.AluOpType.mult,
                                            op1=mybir.AluOpType.add)
                    y = y0i.bitcast(F32)
                    rstd = pp_small.tile([P, 2 * HL], F32, tag="rstd", name="rstd")
                    t_nr = pp_small.tile([P, 2 * HL], F32, tag="t_nr", name="t_nr")
                    cur = y
                    for it in range(2):
                        nc.vector.tensor_mul(t_nr[:], cur, cur)
                        nc.vector.tensor_mul(t_nr[:], t_nr[:], m[:])
                        nc.vector.tensor_scalar(out=t_nr[:], in0=t_nr[:], scalar1=-0.5,
                                                scalar2=1.5, op0=mybir.AluOpType.mult,
                                                op1=mybir.AluOpType.add)
                        nc.vector.tensor_mul(rstd[:], cur, t_nr[:])
                        cur = rstd[:]

                    def finish(x, off, cos_sb, sin_sb, dstT):
                        a = pp.tile([P, HL, D], BF16, tag="a_norm", name="a")
                        rstd_b = bass.AP(
                            tensor=rstd.tensor, offset=rstd.offset + off,
                            ap=[rstd.ap[0], [rstd.ap[1][0], HL], [0, D]],
                        )
                        nc.vector.tensor_mul(a[:], x[:], rstd_b)
                        pre = pp.tile([P, HL, D], BF16, tag="pre", name="pre")
                        nc.vector.tensor_mul(pre[:], a[:], bcast_heads(cos_sb[:, i, :]))
                        tmp = pp.tile([P, HL, RDIM], BF16, tag="tmp_rot", name="tmp")
                        nc.gpsimd.tensor_mul(
                            tmp[:, :, 0:16], a[:, :, 16:32],
                            bcast_heads(sin_sb[:, i, 0:16]),
                        )
                        nc.gpsimd.tensor_mul(
                            tmp[:, :, 16:32], a[:, :, 0:16],
                            bcast_heads(sin_sb[:, i, 16:32]),
                        )
                        nc.gpsimd.tensor_add(
                            pre[:, :, 0:RDIM], pre[:, :, 0:RDIM], tmp[:]
                        )
                        for hp in range(3):
                            nc.sync.dma_start_transpose(
                                out=dstT[hp][i // 4][:, (i % 4) * P : (i % 4 + 1) * P],
                                in_=pre[:, 2 * hp : 2 * hp + 2, :],
                            )

                    finish(xk, 0, kcos_sb, ksin_sb, kT)
                    finish(xq, HL, qcos_sb, qsin_sb, qT)

            for _rep in range(reps):
              # ---------------- pipelined preprocess + attention ----------------
              for qc in range(NQC):
                  for i in range(4 * qc, 4 * qc + 4):
                      preprocess_pair(i)
                  if _rep == 0 and qc == 0:
                      nc.sync.dma_start(out=vall[:], in_=v_t[:])
                      nc.sync.dma_start(out=masks_sb[:], in_=mask_ext[:])
                      nc.sync.dma_start(out=wt_sb[:], in_=wt_ext[:])
                  ypre3 = [ypre_pool.tile([P, 4, 2, D], BF16, tag=f"ypre{_i}", name=f"ypre{_i}")
                           for _i in range(3)]
                  for hp in range(3):
                      ys = [ps_y.tile([P, 4, D + 1], F32, tag="ps_y", name="ys") for _ in range(2)]
                      nkt = 4 * (qc + 1)          # causal k tiles for this q chunk
                      nkg = nkt // 2              # groups of 2 k-tiles
                      for kg in range(nkg):
                        for hh in range(2):
                          hoff = 64 * hh
                          h = 2 * hp + hh
                          ss = ps_s.tile([P, 2, QC], F32, tag="ps_s", name="ss")
                          for j in range(2):
                              kt = kg * 2 + j
                              nc.tensor.matmul(
                                  ss[:, j, :],
                                  kT[hp][kt // 4][hoff : hoff + 64, (kt % 4) * P : (kt % 4 + 1) * P],
                                  qT[hp][qc][hoff : hoff + 64, :],
                                  start=True, stop=True,
                              )
                          pt = pt_pool.tile([P, 2, QC], BF16, tag="pt", name="pt")
                          doff0 = kg * 2 - 4 * qc
                          if False:
                              # far-past unit on DVE: fast exp2 bit-trick
                              # (small attention weights there, approx is safe)
                              t1i = fe_pool.tile([P, 2, QC], mybir.dt.int32,
                                                 tag="t1i", name="t1i")
                              nc.vector.tensor_scalar(
                                  out=t1i[:], in0=ss[:], scalar1=ACON, scalar2=BCON,
                                  op0=mybir.AluOpType.mult, op1=mybir.AluOpType.add)
                              nc.vector.tensor_copy(pt[:], t1i.bitcast(F32))
                          else:
                              nc.scalar.activation(
                                  out=pt[:], in_=ss[:],
                                  func=mybir.ActivationFunctionType.Exp, scale=SC_SCALE,
                              )
                              for j in range(2):
                                  doff = kg * 2 + j - 4 * qc
                                  if doff >= 0:
                                      nc.gpsimd.tensor_mul(
                                          pt[:, j, :],
                                          pt[:, j, :],
                                          masks_sb[:, doff, :],
                                      )
                          for j in range(2):
                              kt = kg * 2 + j
                              for qs in range(4):
                                  first = kt == 0 and qs == 0
                                  last = kt == nkt - 1 and qs == 3
                                  if not first and not last and kt * P >= qc * QC + (qs + 1) * P:
                                      continue
                                  nc.tensor.matmul(
                                      ys[hh][:, qs, :],
                                      pt[:, j, qs * P : (qs + 1) * P],
                                      vall[:, kt, h, :],
                                      start=first, stop=last,
                                  )
                      # normalize: y /= denom (batched over the 4 q subtiles)
                      for hh in range(2):
                          h = 2 * hp + hh
                          r = recip_pool.tile([P, 4], F32, tag="recip", name="r")
                          nc.vector.reciprocal(out=r[:], in_=ys[hh][:, :, D])
                          r_b = bass.AP(tensor=r.tensor, offset=r.offset,
                                        ap=[r.ap[0], r.ap[1], [0, D]])
                          nc.vector.tensor_mul(
                              ypre3[hp][:, :, hh, :],
                              ys[hh][:, :, 0:D], r_b)
                  # y transpose + projection for the 4 finished row tiles
                  for qs in range(4):
                      nt_i = qc * 4 + qs
                      for hp in range(3):
                          nc.sync.dma_start_transpose(
                              out=yT[hp][nt_i][:],
                              in_=ypre3[hp][:, qs, :, :],
                          )
                      osb = outsb_pool.tile([P, E], F32, tag="osb")
                      po = ps_o.tile([P, 2, 512], F32, tag="ps_o", name="po")
                      for oh in range(2):
                          for ec in range(3):
                              nc.tensor.matmul(
                                  po[:, oh, 0:384],
                                  yT[ec][nt_i][:],
                                  wt_sb[:, ec, oh * 384 : (oh + 1) * 384],
                                  start=(ec == 0), stop=(ec == 2),
                              )
                      nc.vector.tensor_copy(osb.rearrange("p (a b) -> p a b", a=2), po[:, :, 0:384])
                      nc.gpsimd.dma_start(out=out_t[nt_i], in_=osb[:])
    return nc


def _get_graph():
    if "nc" not in _CACHE:
        _CACHE["nc"] = build_graph()
    return _CACHE["nc"]


def _host_inputs(q, k, v, q_scale, k_scale, proj_w):
    perm = _head_perm()
    bf = ml_dtypes.bfloat16
    qcos, qsin = _build_tables(q_scale, invert_xpos=False)
    kcos, ksin = _build_tables(k_scale, invert_xpos=True)

    def tab_layout(t):
        # [N, w] -> [128, NT*w] matching sbuf tile [P, NT, w]
        w = t.shape[1]
        return np.ascontiguousarray(
            t.reshape(NT, P, w).transpose(1, 0, 2).reshape(P, NT * w)).astype(bf)

    qcos_r, qsin_r = tab_layout(qcos), tab_layout(qsin)
    kcos_r, ksin_r = tab_layout(kcos), tab_layout(ksin)
    masks = _build_masks()  # [4, 128, 512]
    masks_r = np.ascontiguousarray(
        masks.transpose(1, 0, 2).reshape(P, 4 * QC)).astype(bf)

    in_maps = []
    for c in range(8):
        b = c // 2
        h0 = HL * (c % 2)
        cols = np.concatenate([(h0 + h) * D + perm for h in range(HL)])
        vcols = np.arange(h0 * D, (h0 + HL) * D)
        v_aug = np.ones((N, HL, D + 1), np.float32)
        v_aug[:, :, :D] = v[b][:, vcols].reshape(N, HL, D)
        wt_l = np.ascontiguousarray(proj_w[:, vcols].T)   # [384, 768]
        wt_r = np.ascontiguousarray(
            wt_l.reshape(3, P, E).transpose(1, 0, 2).reshape(P, 3 * E))
        in_maps.append({
            "q": np.ascontiguousarray(q[b][:, cols]).astype(bf),
            "k": np.ascontiguousarray(k[b][:, cols]).astype(bf),
            "v": np.ascontiguousarray(
                v_aug.reshape(NT, P, HL * (D + 1)).transpose(1, 0, 2)
                .reshape(P, NT * HL * (D + 1))).astype(bf),
            "wt": wt_r.astype(bf),
            "qcos": qcos_r, "qsin": qsin_r, "kcos": kcos_r, "ksin": ksin_r,
            "masks": masks_r,
        })
    return in_maps


def kernel(q, k, v, q_scale, k_scale, proj_w, proj_b):
    nc = _get_graph()
    in_maps = _host_inputs(q, k, v, q_scale, k_scale, proj_w)
    res = run_bass_kernel_spmd(nc, in_maps, list(range(8)))
    out = np.empty((B, N, E), np.float32)
    for b in range(B):
        out[b] = res.results[2 * b]["out"] + res.results[2 * b + 1]["out"]
    out += proj_b[None, None, :].astype(np.float32)
    return out



# revision 31
# speedup vs baseline: 1.2166x; 1.2166x over previous
"""Distributed Trainium2 kernel for nn_Attention_64742337020012.

B=4, N=2048, E=768, H=12, D=64 causal attention with per-head RMS norm,
interleaved xpos RoPE, and output projection.

Sharding: 8 cores, core c owns batch c//2 and heads 6*(c%2) .. 6*(c%2)+6
(head-independent attention).  Each core computes full causal attention for
its 6 heads over all 2048 positions plus the partial output projection using
its 384 rows of proj_w^T; the host sums the two partial projections per batch
and adds the bias.

Pipeline (all matmuls bf16, f32 accumulation):
  1. q,k loaded bf16 (cast on host) in batched 4-tile group DMAs, roped on
     DVE via host-precomputed coefficient tables (head dim pre-permuted
     evens-first so the pair swap is two contiguous strided multiply-adds);
     rsqrt(ms) via Quake seed + 2 Newton steps on DVE, batched [P, 48]
     across the 4-tile group (k-side folds 1/sqrt(D):
     rsqrt(ssum+64eps) = 0.125*rstd, so exp needs no extra scale).
  2. q',k' transposed to [d, n] via PE transposes (identity moving matrix)
     into 8 rotating sub-bank PSUM slots, copied to SBUF on DVE.  No DMA
     transposes anywhere.
  3. Scores computed transposed (S^T[k,q] tiles), causal-trimmed moving
     range on diagonal chunks, exp on ACT straight out of PSUM over the
     causally-live column range, triangular mask multiply only on the
     [128,128] diagonal sub-block (DVE).
  4. AV with P^T stationary and [V | 1] moving gives y and the softmax
     denominator in one accumulation; per-row reciprocal normalizes.
     The PE stream is software-pipelined one head deep, with the previous
     head's AV matmuls interleaved between score-matmul groups as filler
     so the PE keeps busy (and its p-state up) while ACT's exp catches up.
  5. y transposed on PE, projected against the core's slice of proj_w^T in
     3x256-col chunks (projection row-tiles spread across head slots so
     PSUM bank recycling hides behind attention work); f32 partial written
     straight from SBUF.
Startup is latency-tuned: one merged rope-table DMA, q/k group loads ahead
of the bulk v/wt loads on the sync queue, and the next q-chunk's
preprocessing emitted inside the current head loop so its PE transposes
never head an in-order stall.
"""

import sys

sys.path.insert(0, "/opt/trn_rl_repo")

import numpy as np
import ml_dtypes

import concourse.bass as bass
import concourse.mybir as mybir
import concourse.tile as tile
from concourse.bass_utils import run_bass_kernel_spmd

# ----------------------------------------------------------------------------
# Workaround for this container's walrus build: the TileContext tail drain
# carries one SyncWait per outstanding semaphore, but CoreV3 CTRL codegen
# accepts only a single sync wait per instruction.  Split the waits across
# single-wait NOPs emitted right after the drain.
from concourse.vector_clock import ScopedClock as _ScopedClock


def _split_sync_waits(nc, inst, max_waits=1):
    si = inst.ins.sync_info
    if si is None:
        return
    waits = list(si.on_wait)
    if len(waits) <= max_waits:
        return
    inst.ins.sync_info = mybir.SyncInfo(
        on_wait=waits[:max_waits], on_update=list(si.on_update)
    )
    for i in range(max_waits, len(waits), max_waits):
        nop = nc.sync.nop(nofuse=True, hint="drain_wait_split")
        nop.ins.sync_info = mybir.SyncInfo(
            on_wait=waits[i : i + max_waits], on_update=[]
        )


def _patched_drain_and_barrier(self, tick_clock, wait_clock):
    nc = self.nc
    drain_inst = nc.sync.drain()
    wait_clock.add_sem_waits(
        drain_inst.ins, _ScopedClock({None: tick_clock.global_clock})
    )
    _split_sync_waits(nc, drain_inst)
    nc.all_engine_barrier()
    assert self.sems is not None
    popped = nc._tile_sem_poison_stack.pop()
    assert popped is self._sem_poison
    nc.clear_and_free_semaphores(list(self.sems.allocated().values()))
    nc.all_engine_barrier()


tile.TileContext._drain_and_barrier = _patched_drain_and_barrier


# Same walrus limitation, applied globally: any instruction carrying more
# than one SyncWait gets the extra waits hoisted onto same-engine NoOps
# inserted immediately before it in the BIR json (equivalent semantics: the
# engine's sequencer performs the waits in sequence).
import json as _json
import concourse.bass2jax as _bass2jax

_orig_compile_bir_kernel = _bass2jax.compile_bir_kernel


def _split_waits_in_bir(bir_json: bytes) -> bytes:
    j = _json.loads(bir_json)
    n_new = [0]
    for fn in j["functions"]:
        for bb in fn["blocks"]:
            insts = bb["instructions"]
            out = []
            for inst in insts:
                si = inst.get("sync_info")
                waits = (si or {}).get("on_wait") or []
                if len(waits) > 1:
                    for w in waits[:-1]:
                        n_new[0] += 1
                        out.append({
                            "engine": inst["engine"],
                            "ins": [], "outs": [],
                            "name": f"{inst['name']}-ws{n_new[0]}",
                            "opcode": "NoOp",
                            "sync_info": {"on_wait": [w], "on_update": []},
                        })
                    si["on_wait"] = [waits[-1]]
                out.append(inst)
            bb["instructions"] = out
    return _json.dumps(j).encode()


def _patched_compile_bir_kernel(bir_json, tmpdir, neff_name="file.neff"):
    return _orig_compile_bir_kernel(_split_waits_in_bir(bir_json), tmpdir, neff_name)


_bass2jax.compile_bir_kernel = _patched_compile_bir_kernel
# ----------------------------------------------------------------------------

B, N, E, H = 4, 2048, 768, 12
D = 64
RDIM = 32
EPS = 1e-6
XPOS_SCALE_BASE = 512.0
THETA = 10000.0

HL = 6            # heads per core
EL = HL * D       # 384 local embed cols
P = 128
NT = N // P       # 16 row tiles
QC = 512          # q chunk (columns of S^T tiles)
NQC = N // QC     # 4
F32 = mybir.dt.float32
BF16 = mybir.dt.bfloat16
I32 = mybir.dt.int32

_CACHE = {}


def _head_perm():
    """Per-head column permutation: rotary evens, rotary odds, passthrough."""
    p = list(range(0, RDIM, 2)) + list(range(1, RDIM, 2)) + list(range(RDIM, D))
    return np.array(p, dtype=np.int64)


def _build_tables(scale_vec, invert_xpos):
    """cosPt [N, 64], sinPt [N, 32] coefficient tables in permuted layout.

    slot i (i<16):    out = x_e[i]*cosPt[i] + x_o[i]*sinPt[i]
    slot 16+i:        out = x_o[i]*cosPt[16+i] + x_e[i]*sinPt[16+i]
    slot 32+j:        out = x[32+j]*cosPt[32+j]
    scale_vec: rms scale (q_scale/k_scale), indexed in ORIGINAL layout.
    Returns (cosPt, sinPt) including the xpos scale (inverted for k).
    """
    inv_freq = 1.0 / (THETA ** (np.arange(0, RDIM, 2, dtype=np.float64) / RDIM))
    t = np.arange(N, dtype=np.float64)
    freqs = t[:, None] * inv_freq[None, :]           # [N, 16]
    cos0, sin0 = np.cos(freqs), np.sin(freqs)
    base = (np.arange(0, RDIM, 2, dtype=np.float64) + 0.4 * RDIM) / (1.4 * RDIM)
    power = (t - N // 2) / XPOS_SCALE_BASE
    xsc = base[None, :] ** power[:, None]            # [N, 16]
    if invert_xpos:
        xsc = 1.0 / xsc
    sc = np.asarray(scale_vec, dtype=np.float64)
    cosPt = np.empty((N, D), dtype=np.float64)
    sinPt = np.empty((N, RDIM), dtype=np.float64)
    i = np.arange(16)
    cosPt[:, 0:16] = cos0 * xsc * sc[2 * i][None, :]
    cosPt[:, 16:32] = cos0 * xsc * sc[2 * i + 1][None, :]
    cosPt[:, 32:] = sc[RDIM:][None, :]
    sinPt[:, 0:16] = -sin0 * xsc * sc[2 * i + 1][None, :]
    sinPt[:, 16:32] = sin0 * xsc * sc[2 * i][None, :]
    return cosPt.astype(np.float32), sinPt.astype(np.float32)


def build_graph():
    nc = bass.Bass()
    q_ext = nc.declare_dram_parameter("q", [N, EL], BF16, isOutput=False)
    k_ext = nc.declare_dram_parameter("k", [N, EL], BF16, isOutput=False)
    v_ext = nc.declare_dram_parameter("v", [P, NT * HL * (D + 1)], BF16, isOutput=False)
    wt_ext = nc.declare_dram_parameter("wt", [P, 3 * E], BF16, isOutput=False)
    tabs_ext = nc.declare_dram_parameter(
        "tabs", [P, NT * 2 * (D + RDIM)], BF16, isOutput=False)
    tri_ext = nc.declare_dram_parameter("tri", [P, P], BF16, isOutput=False)
    out_ext = nc.declare_dram_parameter("out", [N, E], F32, isOutput=True)

    q_t4 = q_ext.rearrange("(g t p) e -> g p t e", t=4, p=P)
    k_t4 = k_ext.rearrange("(g t p) e -> g p t e", t=4, p=P)
    out_t = out_ext.rearrange("(t p) e -> t p e", p=P)

    def bcast_heads(ap, nh=HL):
        return bass.AP(tensor=ap.tensor, offset=ap.offset,
                       ap=[ap.ap[0], [0, nh], ap.ap[1]])

    with tile.TileContext(nc) as tc:
        with (
            tc.tile_pool(name="persist", bufs=1) as persist,
            tc.tile_pool(name="qk_in", bufs=10) as qk_in,
            tc.tile_pool(name="pp", bufs=8) as pp,
            tc.tile_pool(name="pp_small", bufs=4) as pp_small,
            tc.tile_pool(name="pt_pool", bufs=20) as pt_pool,
            tc.tile_pool(name="ypre", bufs=3) as ypre_pool,
            tc.tile_pool(name="yt_pool", bufs=3) as yt_pool,
            tc.tile_pool(name="recip", bufs=8) as recip_pool,
            tc.tile_pool(name="outsb", bufs=4) as outsb_pool,
            tc.tile_pool(name="ps_s", bufs=2, space="PSUM") as ps_s,
            tc.tile_pool(name="ps_y", bufs=2, space="PSUM") as ps_y,
            tc.tile_pool(name="ps_t", bufs=1, space="PSUM") as ps_t,
            tc.tile_pool(name="ps_o", bufs=1, space="PSUM") as ps_o,
        ):
            # ---------------- constants (host-prepared layouts) ----------------
            ident = persist.tile([P, P], BF16)
            from concourse.masks import make_identity
            make_identity(nc, ident)
            tabs_sb = persist.tile([P, NT, 2, D + RDIM], BF16)
            tri_sb = persist.tile([P, P], BF16)
            wt_sb = persist.tile([P, 3, E], BF16)
            vall = persist.tile([P, NT, HL, D + 1], BF16)
            # rope tables + tri on the sync queue (critical path: first
            # preprocess / first diag mask); bulk v/wt go on sync AFTER the
            # startup q/k tile loads (below).
            nc.sync.dma_start(out=tabs_sb[:], in_=tabs_ext[:])
            nc.sync.dma_start(out=tri_sb[:], in_=tri_ext[:])
            kcos_sb = tabs_sb[:, :, 0, 0:D]
            ksin_sb = tabs_sb[:, :, 0, D:]
            qcos_sb = tabs_sb[:, :, 1, 0:D]
            qsin_sb = tabs_sb[:, :, 1, D:]


            # transposed q', k': [128 = 2-head d, hp, n]
            qT = persist.tile([P, 3, N], BF16, name="qT")
            kT = persist.tile([P, 3, N], BF16, name="kT")

            # rotating PSUM transpose slots: 8 x [P, 128] bf16 in one bank
            psT = ps_t.tile([P, 8, P], BF16, name="psT")
            slot_ctr = [0]

            def next_slot():
                s = slot_ctr[0] % 8
                slot_ctr[0] += 1
                return psT[:, s, :]

            # ---------------- preprocess: 4-tile groups ----------------
            # The Newton-rsqrt chain runs once per GROUP of 4 tiles on
            # [P, 48] batches (12 DVE ops instead of 48), cutting both DVE
            # load and the startup latency to the first transposed q/k.
            def preprocess_group(i0):
                g = i0 // 4
                ssum4 = pp_small.tile([P, 4, 2 * HL], F32, tag="ssum4",
                                      name="ssum4")
                xg = qk_in.tile([P, 2, 4, HL, D], BF16, tag="xqk", name="x")
                nc.sync.dma_start(out=xg[:, 0], in_=k_t4[g])
                nc.sync.dma_start(out=xg[:, 1], in_=q_t4[g])
                for ii in range(4):
                    sq = pp.tile([P, 2, HL, D], BF16, tag="sq", name="sq")
                    nc.gpsimd.tensor_mul(sq[:], xg[:, :, ii], xg[:, :, ii])
                    nc.vector.reduce_sum(ssum4[:, ii, :], sq[:],
                                         axis=mybir.AxisListType.X)
                # rstd via Quake seed + 2 Newton steps (DVE, batched x4).
                # k half: rsqrt(ssum + 64*eps) = 0.125 * rstd_k (folds 1/sqrt(D))
                # q half: rsqrt(ssum/64 + eps) = rstd_q
                m = pp_small.tile([P, 4, 2 * HL], F32, tag="m_ms", name="m")
                nc.vector.tensor_scalar(out=m[:, :, 0:HL],
                                        in0=ssum4[:, :, 0:HL],
                                        scalar1=float(D) * EPS, scalar2=None,
                                        op0=mybir.AluOpType.add)
                nc.vector.tensor_scalar(out=m[:, :, HL:],
                                        in0=ssum4[:, :, HL:],
                                        scalar1=1.0 / D, scalar2=EPS,
                                        op0=mybir.AluOpType.mult,
                                        op1=mybir.AluOpType.add)
                ish = pp_small.tile([P, 4, 2 * HL], I32, tag="ish", name="ish")
                nc.vector.tensor_scalar(out=ish[:], in0=m.bitcast(I32),
                                        scalar1=1, scalar2=None,
                                        op0=mybir.AluOpType.logical_shift_right)
                y0i = pp_small.tile([P, 4, 2 * HL], I32, tag="y0i", name="y0i")
                nc.vector.tensor_scalar(out=y0i[:], in0=ish[:],
                                        scalar1=-1, scalar2=0x5F3759DF,
                                        op0=mybir.AluOpType.mult,
                                        op1=mybir.AluOpType.add)
                y = y0i.bitcast(F32)
                rstd = pp_small.tile([P, 4, 2 * HL], F32, tag="rstd", name="rstd")
                t_nr = pp_small.tile([P, 4, 2 * HL], F32, tag="t_nr", name="t_nr")
                cur = y
                for _it in range(2):
                    nc.vector.tensor_mul(t_nr[:], cur, cur)
                    nc.vector.tensor_mul(t_nr[:], t_nr[:], m[:])
                    nc.vector.tensor_scalar(out=t_nr[:], in0=t_nr[:], scalar1=-0.5,
                                            scalar2=1.5, op0=mybir.AluOpType.mult,
                                            op1=mybir.AluOpType.add)
                    nc.vector.tensor_mul(rstd[:], cur, t_nr[:])
                    cur = rstd[:]
                return xg, rstd

            def finish_tile(grp, ii, i):
                xg, rstd = grp

                def finish(xh, off, cos_sb, sin_sb, dstT):
                    # rope on raw x (rstd commutes with the rotation)
                    pre = pp.tile([P, HL, D], BF16, tag="pre", name="pre")
                    nc.vector.tensor_mul(pre[:], xh, bcast_heads(cos_sb[:, i]))
                    tmp = pp.tile([P, HL, RDIM], BF16, tag="tmp_rot", name="tmp")
                    nc.gpsimd.tensor_mul(
                        tmp[:, :, 0:16], xh[:, :, 16:32],
                        bcast_heads(sin_sb[:, i, 0:16]),
                    )
                    nc.gpsimd.tensor_mul(
                        tmp[:, :, 16:32], xh[:, :, 0:16],
                        bcast_heads(sin_sb[:, i, 16:32]),
                    )
                    nc.gpsimd.tensor_add(
                        pre[:, :, 0:RDIM], pre[:, :, 0:RDIM], tmp[:]
                    )
                    # apply rstd (per n,head scalar, broadcast along d)
                    a = pp.tile([P, HL, D], BF16, tag="a_norm", name="a")
                    rstd_b = bass.AP(
                        tensor=rstd.tensor,
                        offset=rstd.offset + ii * 2 * HL + off,
                        ap=[rstd.ap[0], [1, HL], [0, D]],
                    )
                    nc.vector.tensor_mul(a[:], pre[:], rstd_b)
                    # PE transpose per head pair, copy to [d, n] SBUF
                    for hp in range(3):
                        slot = next_slot()
                        nc.tensor.transpose(
                            slot, a[:, 2 * hp : 2 * hp + 2, :], ident[:]
                        )
                        nc.vector.tensor_copy(
                            dstT[:, hp, i * P : (i + 1) * P], slot
                        )

                finish(xg[:, 0, ii], 0, kcos_sb, ksin_sb, kT)
                finish(xg[:, 1, ii], HL, qcos_sb, qsin_sb, qT)

            # ---------------- attention emit helpers ----------------
            def emit_qk_exp(h, qc, filler):
                """Scores + exp + diag mask for (head, q-chunk). Returns pt list."""
                hp, hh = divmod(h, 2)
                hoff = 64 * hh
                nkt = 4 * (qc + 1)
                ngroups = nkt // 2
                pts = []
                for kg in range(ngroups):
                    ss = ps_s.tile([P, 2, QC], F32, tag="ps_s", name="ss")
                    for j in range(2):
                        kt = kg * 2 + j
                        doff = kt - 4 * qc
                        qstart = max(doff, 0) * P
                        nc.tensor.matmul(
                            ss[:, j, qstart:QC],
                            kT[hoff : hoff + 64, hp, kt * P : (kt + 1) * P],
                            qT[hoff : hoff + 64, hp, qc * QC + qstart : (qc + 1) * QC],
                            start=True, stop=True,
                        )
                    pt = pt_pool.tile([P, 2, QC], BF16, tag="pt", name="pt")
                    # exp only the causally-live q range (union over the pair)
                    qmin = max(kg * 2 - 4 * qc, 0) * P
                    nc.scalar.activation(
                        out=pt[:, :, qmin:], in_=ss[:, :, qmin:],
                        func=mybir.ActivationFunctionType.Exp,
                    )
                    for j in range(2):
                        doff = kg * 2 + j - 4 * qc
                        if doff >= 0:
                            nc.vector.tensor_mul(
                                pt[:, j, doff * P : (doff + 1) * P],
                                pt[:, j, doff * P : (doff + 1) * P],
                                tri_sb[:],
                            )
                    pts.append(pt)
                    # interleave prev head's AV units as PE filler
                    if filler:
                        n = -(-len(filler) // (ngroups - kg))
                        for _ in range(min(n, len(filler))):
                            filler.pop(0)()
                return pts

            def make_av_units(h, qc, pts, ypre3):
                """Per-kt AV emission closures + final normalize closure.

                The caller interleaves these between the next head's QK
                groups so the PE always has ready-to-run filler work while
                exp catches up (keeps the p-state ramped).
                """
                hp, hh = divmod(h, 2)
                nkt = 4 * (qc + 1)
                ys = ps_y.tile([P, 4, D + 1], F32, tag="ps_y", name="ys")

                def mk(kt):
                    def unit():
                        pt = pts[kt // 2]
                        j = kt % 2
                        for qs in range(4):
                            first = kt == 0 and qs == 0
                            last = kt == nkt - 1 and qs == 3
                            if (not first and not last
                                    and kt * P >= qc * QC + (qs + 1) * P):
                                continue
                            nc.tensor.matmul(
                                ys[:, qs, :],
                                pt[:, j, qs * P : (qs + 1) * P],
                                vall[:, kt, h, :],
                                start=first, stop=last,
                            )
                    return unit

                def fin():
                    r = recip_pool.tile([P, 4], F32, tag="recip", name="r")
                    nc.vector.reciprocal(out=r[:], in_=ys[:, :, D])
                    r_b = bass.AP(tensor=r.tensor, offset=r.offset,
                                  ap=[r.ap[0], r.ap[1], [0, D]])
                    nc.vector.tensor_mul(
                        ypre3[hp][:, :, hh, :], ys[:, :, 0:D], r_b)

                return [mk(kt) for kt in range(nkt)] + [fin]

            def emit_proj_nt(qc, ypre3, qs):
                """y transpose + projection + output store for one row tile."""
                nt_i = qc * 4 + qs
                yt = yt_pool.tile([P, 3, P], BF16, tag="yt", name="yt")
                for hp in range(3):
                    slot = next_slot()
                    nc.tensor.transpose(
                        slot, ypre3[hp][:, qs, :, :], ident[:]
                    )
                    nc.vector.tensor_copy(yt[:, hp, :], slot)
                osb = outsb_pool.tile([P, E], F32, tag="osb")
                po = ps_o.tile([P, 2, 256], F32, tag="ps_o", name="po")
                for oh in range(3):
                    sl = oh % 2
                    for ec in range(3):
                        nc.tensor.matmul(
                            po[:, sl, :],
                            yt[:, ec, :],
                            wt_sb[:, ec, oh * 256 : (oh + 1) * 256],
                            start=(ec == 0), stop=(ec == 2),
                        )
                    nc.vector.tensor_copy(
                        osb[:, oh * 256 : (oh + 1) * 256], po[:, sl, :]
                    )
                nc.sync.dma_start(out=out_t[nt_i], in_=osb[:])

            # ---------------- main pipelined schedule ----------------
            pending_av = []     # at most one (h, qc, pts, ypre3)
            pending_proj = []   # (qc, ypre3, qs) row-tile units
            ypre3_cur = None
            grp = preprocess_group(0)
            for ii in range(4):
                finish_tile(grp, ii, ii)
            # v/wt dispatched on sync AFTER the startup q/k tile loads so the
            # small tile transfers aren't queued behind 3.8MB of bulk data
            nc.sync.dma_start(out=vall[:], in_=v_ext[:])
            nc.sync.dma_start(out=wt_sb[:], in_=wt_ext[:])
            for qc in range(NQC):
                ypre3_cur = [
                    ypre_pool.tile([P, 4, 2, D], BF16, tag=f"ypre{_i}",
                                   name=f"ypre{_i}")
                    for _i in range(3)
                ]
                for h in range(HL):
                    filler = []
                    meta = None
                    if pending_av:
                        ph, pqc, ppts, pypre = pending_av.pop()
                        filler = make_av_units(ph, pqc, ppts, pypre)
                        meta = (ph, pqc, pypre)
                    pts = emit_qk_exp(h, qc, filler)
                    for u in filler:
                        u()
                    if meta is not None:
                        ph, pqc, pypre = meta
                        if ph == HL - 1:
                            pending_proj.extend(
                                (pqc, pypre, qs) for qs in range(4))
                        elif pending_proj:
                            emit_proj_nt(*pending_proj.pop(0))
                    pending_av.append((h, qc, pts, ypre3_cur))
                    # preprocess next q-chunk's tiles inside the head loop so
                    # their PE transposes never head an in-order stall at the
                    # chunk boundary
                    if qc < NQC - 1:
                        if h == 1:
                            grp = preprocess_group(4 * (qc + 1))
                        elif 2 <= h:
                            finish_tile(grp, h - 2, 4 * (qc + 1) + (h - 2))
            # drain
            ph, pqc, ppts, pypre = pending_av.pop()
            for u in make_av_units(ph, pqc, ppts, pypre):
                u()
            while pending_proj:
                emit_proj_nt(*pending_proj.pop(0))
            for qs in range(4):
                emit_proj_nt(pqc, pypre, qs)
    return nc


def _get_graph():
    if "nc" not in _CACHE:
        _CACHE["nc"] = build_graph()
    return _CACHE["nc"]


def _host_inputs(q, k, v, q_scale, k_scale, proj_w):
    perm = _head_perm()
    bf = ml_dtypes.bfloat16
    qcos, qsin = _build_tables(q_scale, invert_xpos=False)
    kcos, ksin = _build_tables(k_scale, invert_xpos=True)

    def tab_layout(t):
        # [N, w] -> [128, NT*w] matching sbuf tile [P, NT, w]
        w = t.shape[1]
        return np.ascontiguousarray(
            t.reshape(NT, P, w).transpose(1, 0, 2).reshape(P, NT * w)).astype(bf)

    qcos_r, qsin_r = tab_layout(qcos), tab_layout(qsin)
    kcos_r, ksin_r = tab_layout(kcos), tab_layout(ksin)
    # combined [P, NT, 2, D+RDIM]: slot 0 = k tables, slot 1 = q tables
    tabs = np.empty((P, NT, 2, D + RDIM), dtype=kcos_r.dtype)
    tabs[:, :, 0, 0:D] = kcos_r.reshape(P, NT, D)
    tabs[:, :, 0, D:] = ksin_r.reshape(P, NT, RDIM)
    tabs[:, :, 1, 0:D] = qcos_r.reshape(P, NT, D)
    tabs[:, :, 1, D:] = qsin_r.reshape(P, NT, RDIM)
    tabs_r = np.ascontiguousarray(tabs.reshape(P, NT * 2 * (D + RDIM)))
    # triangular mask for the diagonal [128,128] sub-block: tri[k, q] = q >= k
    tri = (np.arange(P)[None, :] >= np.arange(P)[:, None]).astype(np.float32)
    tri_r = np.ascontiguousarray(tri).astype(bf)

    in_maps = []
    for c in range(8):
        b = c // 2
        h0 = HL * (c % 2)
        cols = np.concatenate([(h0 + h) * D + perm for h in range(HL)])
        vcols = np.arange(h0 * D, (h0 + HL) * D)
        v_aug = np.ones((N, HL, D + 1), np.float32)
        v_aug[:, :, :D] = v[b][:, vcols].reshape(N, HL, D)
        wt_l = np.ascontiguousarray(proj_w[:, vcols].T)   # [384, 768]
        wt_r = np.ascontiguousarray(
            wt_l.reshape(3, P, E).transpose(1, 0, 2).reshape(P, 3 * E))
        in_maps.append({
            "q": np.ascontiguousarray(q[b][:, cols]).astype(bf),
            "k": np.ascontiguousarray(k[b][:, cols]).astype(bf),
            "v": np.ascontiguousarray(
                v_aug.reshape(NT, P, HL * (D + 1)).transpose(1, 0, 2)
                .reshape(P, NT * HL * (D + 1))).astype(bf),
            "wt": wt_r.astype(bf),
            "tabs": tabs_r,
            "tri": tri_r,
        })
    return in_maps


def kernel(q, k, v, q_scale, k_scale, proj_w, proj_b):
    nc = _get_graph()
    in_maps = _host_inputs(q, k, v, q_scale, k_scale, proj_w)
    res = run_bass_kernel_spmd(nc, in_maps, list(range(8)))
    out = np.empty((B, N, E), np.float32)
    for b in range(B):
        out[b] = res.results[2 * b]["out"] + res.results[2 * b + 1]["out"]
    out += proj_b[None, None, :].astype(np.float32)
    return out


# revision 33
# speedup vs baseline: 1.5296x; 1.2573x over previous
"""Distributed Trainium2 kernel for nn_Attention_64742337020012.

B=4, N=2048, E=768, H=12, D=64 causal attention with per-head RMS norm,
interleaved xpos RoPE, and output projection.

Sharding: 8 cores, core c owns batch c//2 and heads 6*(c%2) .. 6*(c%2)+6
(head-independent attention).  Each core computes full causal attention for
its 6 heads over all 2048 positions plus the partial output projection using
its 384 rows of proj_w^T; the host sums the two partial projections per batch
and adds the bias.

Pipeline (all matmuls bf16, f32 accumulation):
  1. q,k loaded bf16 (cast on host) in batched 4-tile group DMAs, roped on
     DVE via host-precomputed coefficient tables (head dim pre-permuted
     evens-first so the pair swap is two contiguous strided multiply-adds);
     rsqrt(ms) via Quake seed + 2 Newton steps on DVE, batched [P, 48]
     across the 4-tile group (k-side folds 1/sqrt(D):
     rsqrt(ssum+64eps) = 0.125*rstd, so exp needs no extra scale).
  2. q',k' transposed to [d, n] via PE transposes (identity moving matrix)
     into 8 rotating sub-bank PSUM slots, copied to SBUF on DVE.  No DMA
     transposes anywhere.
  3. Scores computed transposed (S^T[k,q] tiles), causal-trimmed moving
     range on diagonal chunks, exp on ACT straight out of PSUM over the
     causally-live column range, triangular mask multiply only on the
     [128,128] diagonal sub-block (DVE).
  4. AV with P^T stationary and [V | 1] moving gives y and the softmax
     denominator in one accumulation; per-row reciprocal normalizes.
     The PE stream is software-pipelined one head deep, with the previous
     head's AV matmuls interleaved between score-matmul groups as filler
     so the PE keeps busy (and its p-state up) while ACT's exp catches up.
  5. y transposed on PE, projected against the core's slice of proj_w^T in
     3x256-col chunks (projection row-tiles spread across head slots so
     PSUM bank recycling hides behind attention work); f32 partial written
     straight from SBUF.
Startup is latency-tuned: one merged rope-table DMA, q/k group loads ahead
of the bulk v/wt loads on the sync queue, and the next q-chunk's
preprocessing emitted inside the current head loop so its PE transposes
never head an in-order stall.
"""

import sys

sys.path.insert(0, "/opt/trn_rl_repo")

import numpy as np
import ml_dtypes

import concourse.bass as bass
import concourse.mybir as mybir
import concourse.tile as tile
from concourse.bass_utils import run_bass_kernel_spmd

# ----------------------------------------------------------------------------
# Workaround for this container's walrus build: the TileContext tail drain
# carries one SyncWait per outstanding semaphore, but CoreV3 CTRL codegen
# accepts only a single sync wait per instruction.  Split the waits across
# single-wait NOPs emitted right after the drain.
from concourse.vector_clock import ScopedClock as _ScopedClock


def _split_sync_waits(nc, inst, max_waits=1):
    si = inst.ins.sync_info
    if si is None:
        return
    waits = list(si.on_wait)
    if len(waits) <= max_waits:
        return
    inst.ins.sync_info = mybir.SyncInfo(
        on_wait=waits[:max_waits], on_update=list(si.on_update)
    )
    for i in range(max_waits, len(waits), max_waits):
        nop = nc.sync.nop(nofuse=True, hint="drain_wait_split")
        nop.ins.sync_info = mybir.SyncInfo(
            on_wait=waits[i : i + max_waits], on_update=[]
        )


def _patched_drain_and_barrier(self, tick_clock, wait_clock):
    nc = self.nc
    drain_inst = nc.sync.drain()
    wait_clock.add_sem_waits(
        drain_inst.ins, _ScopedClock({None: tick_clock.global_clock})
    )
    _split_sync_waits(nc, drain_inst)
    nc.all_engine_barrier()
    assert self.sems is not None
    popped = nc._tile_sem_poison_stack.pop()
    assert popped is self._sem_poison
    nc.clear_and_free_semaphores(list(self.sems.allocated().values()))
    nc.all_engine_barrier()


tile.TileContext._drain_and_barrier = _patched_drain_and_barrier


# Same walrus limitation, applied globally: any instruction carrying more
# than one SyncWait gets the extra waits hoisted onto same-engine NoOps
# inserted immediately before it in the BIR json (equivalent semantics: the
# engine's sequencer performs the waits in sequence).
import json as _json
import concourse.bass2jax as _bass2jax

_orig_compile_bir_kernel = _bass2jax.compile_bir_kernel


def _split_waits_in_bir(bir_json: bytes) -> bytes:
    j = _json.loads(bir_json)
    n_new = [0]
    for fn in j["functions"]:
        for bb in fn["blocks"]:
            insts = bb["instructions"]
            out = []
            for inst in insts:
                si = inst.get("sync_info")
                waits = (si or {}).get("on_wait") or []
                if len(waits) > 1:
                    for w in waits[:-1]:
                        n_new[0] += 1
                        out.append({
                            "engine": inst["engine"],
                            "ins": [], "outs": [],
                            "name": f"{inst['name']}-ws{n_new[0]}",
                            "opcode": "NoOp",
                            "sync_info": {"on_wait": [w], "on_update": []},
                        })
                    si["on_wait"] = [waits[-1]]
                out.append(inst)
            bb["instructions"] = out
    return _json.dumps(j).encode()


def _patched_compile_bir_kernel(bir_json, tmpdir, neff_name="file.neff"):
    return _orig_compile_bir_kernel(_split_waits_in_bir(bir_json), tmpdir, neff_name)


_bass2jax.compile_bir_kernel = _patched_compile_bir_kernel
# ----------------------------------------------------------------------------

B, N, E, H = 4, 2048, 768, 12
D = 64
RDIM = 32
EPS = 1e-6
XPOS_SCALE_BASE = 512.0
THETA = 10000.0

HL = 6            # heads per core
EL = HL * D       # 384 local embed cols
P = 128
NT = N // P       # 16 row tiles
QC = 512          # q chunk (columns of S^T tiles)
NQC = N // QC     # 4
F32 = mybir.dt.float32
BF16 = mybir.dt.bfloat16
I32 = mybir.dt.int32

_CACHE = {}


def _head_perm():
    """Per-head column permutation: rotary evens, rotary odds, passthrough."""
    p = list(range(0, RDIM, 2)) + list(range(1, RDIM, 2)) + list(range(RDIM, D))
    return np.array(p, dtype=np.int64)


def _build_tables(scale_vec, invert_xpos):
    """cosPt [N, 64], sinPt [N, 32] coefficient tables in permuted layout.

    slot i (i<16):    out = x_e[i]*cosPt[i] + x_o[i]*sinPt[i]
    slot 16+i:        out = x_o[i]*cosPt[16+i] + x_e[i]*sinPt[16+i]
    slot 32+j:        out = x[32+j]*cosPt[32+j]
    scale_vec: rms scale (q_scale/k_scale), indexed in ORIGINAL layout.
    Returns (cosPt, sinPt) including the xpos scale (inverted for k).
    """
    inv_freq = 1.0 / (THETA ** (np.arange(0, RDIM, 2, dtype=np.float64) / RDIM))
    t = np.arange(N, dtype=np.float64)
    freqs = t[:, None] * inv_freq[None, :]           # [N, 16]
    cos0, sin0 = np.cos(freqs), np.sin(freqs)
    base = (np.arange(0, RDIM, 2, dtype=np.float64) + 0.4 * RDIM) / (1.4 * RDIM)
    power = (t - N // 2) / XPOS_SCALE_BASE
    xsc = base[None, :] ** power[:, None]            # [N, 16]
    if invert_xpos:
        xsc = 1.0 / xsc
    sc = np.asarray(scale_vec, dtype=np.float64)
    cosPt = np.empty((N, D), dtype=np.float64)
    sinPt = np.empty((N, RDIM), dtype=np.float64)
    i = np.arange(16)
    cosPt[:, 0:16] = cos0 * xsc * sc[2 * i][None, :]
    cosPt[:, 16:32] = cos0 * xsc * sc[2 * i + 1][None, :]
    cosPt[:, 32:] = sc[RDIM:][None, :]
    sinPt[:, 0:16] = -sin0 * xsc * sc[2 * i + 1][None, :]
    sinPt[:, 16:32] = sin0 * xsc * sc[2 * i][None, :]
    return cosPt.astype(np.float32), sinPt.astype(np.float32)


def build_graph():
    nc = bass.Bass()
    q_ext = nc.declare_dram_parameter("q", [N, EL], BF16, isOutput=False)
    k_ext = nc.declare_dram_parameter("k", [N, EL], BF16, isOutput=False)
    v_ext = nc.declare_dram_parameter("v", [P, NT * HL * (D + 1)], BF16, isOutput=False)
    wt_ext = nc.declare_dram_parameter("wt", [P, 3 * E], BF16, isOutput=False)
    tabs_ext = nc.declare_dram_parameter(
        "tabs", [P, NT * 2 * (D + RDIM)], BF16, isOutput=False)
    tri_ext = nc.declare_dram_parameter("tri", [P, P], BF16, isOutput=False)
    out_ext = nc.declare_dram_parameter("out", [N, E], F32, isOutput=True)

    q_t4 = q_ext.rearrange("(g t p) e -> g p t e", t=4, p=P)
    k_t4 = k_ext.rearrange("(g t p) e -> g p t e", t=4, p=P)
    out_t = out_ext.rearrange("(t p) e -> t p e", p=P)

    def bcast_heads(ap, nh=HL):
        return bass.AP(tensor=ap.tensor, offset=ap.offset,
                       ap=[ap.ap[0], [0, nh], ap.ap[1]])

    with tile.TileContext(nc) as tc:
        with (
            tc.tile_pool(name="persist", bufs=1) as persist,
            tc.tile_pool(name="qk_in", bufs=10) as qk_in,
            tc.tile_pool(name="pp", bufs=8) as pp,
            tc.tile_pool(name="pp_small", bufs=4) as pp_small,
            tc.tile_pool(name="pt_pool", bufs=20) as pt_pool,
            tc.tile_pool(name="ypre", bufs=3) as ypre_pool,
            tc.tile_pool(name="yt_pool", bufs=3) as yt_pool,
            tc.tile_pool(name="recip", bufs=8) as recip_pool,
            tc.tile_pool(name="outsb", bufs=4) as outsb_pool,
            tc.tile_pool(name="ps_s", bufs=2, space="PSUM") as ps_s,
            tc.tile_pool(name="ps_y", bufs=2, space="PSUM") as ps_y,
            tc.tile_pool(name="ps_t", bufs=1, space="PSUM") as ps_t,
            tc.tile_pool(name="ps_o", bufs=1, space="PSUM") as ps_o,
        ):
            # ---------------- constants (host-prepared layouts) ----------------
            ident = persist.tile([P, P], BF16)
            from concourse.masks import make_identity
            make_identity(nc, ident)
            tabs_sb = persist.tile([P, NT, 2, D + RDIM], BF16)
            tri_sb = persist.tile([P, P], BF16)
            wt_sb = persist.tile([P, 3, E], BF16)
            vall = persist.tile([P, NT, HL, D + 1], BF16)
            # rope tables + tri on the sync queue (critical path: first
            # preprocess / first diag mask); bulk v/wt go on sync AFTER the
            # startup q/k tile loads (below).
            nc.sync.dma_start(out=tabs_sb[:], in_=tabs_ext[:])
            nc.sync.dma_start(out=tri_sb[:], in_=tri_ext[:])
            kcos_sb = tabs_sb[:, :, 0, 0:D]
            ksin_sb = tabs_sb[:, :, 0, D:]
            qcos_sb = tabs_sb[:, :, 1, 0:D]
            qsin_sb = tabs_sb[:, :, 1, D:]


            # transposed q', k': [128 = 2-head d, hp, n]
            qT = persist.tile([P, 3, N], BF16, name="qT")
            kT = persist.tile([P, 3, N], BF16, name="kT")

            # rotating PSUM transpose slots: 8 x [P, 128] bf16 in one bank
            psT = ps_t.tile([P, 8, P], BF16, name="psT")
            slot_ctr = [0]

            def next_slot():
                s = slot_ctr[0] % 8
                slot_ctr[0] += 1
                return psT[:, s, :]

            # ---------------- preprocess: 4-tile groups ----------------
            # The Newton-rsqrt chain runs once per GROUP of 4 tiles on
            # [P, 48] batches (12 DVE ops instead of 48), cutting both DVE
            # load and the startup latency to the first transposed q/k.
            def preprocess_group(i0, sq_eng=None):
                g = i0 // 4
                sq_eng = sq_eng or nc.gpsimd
                ssum4 = pp_small.tile([P, 4, 2 * HL], F32, tag="ssum4",
                                      name="ssum4")
                xg = qk_in.tile([P, 2, 4, HL, D], BF16, tag="xqk", name="x")
                nc.sync.dma_start(out=xg[:, 0], in_=k_t4[g])
                nc.sync.dma_start(out=xg[:, 1], in_=q_t4[g])
                for ii in range(4):
                    sq = pp.tile([P, 2, HL, D], BF16, tag="sq", name="sq")
                    sq_eng.tensor_mul(sq[:], xg[:, :, ii], xg[:, :, ii])
                    nc.vector.reduce_sum(ssum4[:, ii, :], sq[:],
                                         axis=mybir.AxisListType.X)
                # rstd via Quake seed + 2 Newton steps (DVE, batched x4).
                # k half: rsqrt(ssum + 64*eps) = 0.125 * rstd_k (folds 1/sqrt(D))
                # q half: rsqrt(ssum/64 + eps) = rstd_q
                m = pp_small.tile([P, 4, 2 * HL], F32, tag="m_ms", name="m")
                nc.vector.tensor_scalar(out=m[:, :, 0:HL],
                                        in0=ssum4[:, :, 0:HL],
                                        scalar1=float(D) * EPS, scalar2=None,
                                        op0=mybir.AluOpType.add)
                nc.vector.tensor_scalar(out=m[:, :, HL:],
                                        in0=ssum4[:, :, HL:],
                                        scalar1=1.0 / D, scalar2=EPS,
                                        op0=mybir.AluOpType.mult,
                                        op1=mybir.AluOpType.add)
                ish = pp_small.tile([P, 4, 2 * HL], I32, tag="ish", name="ish")
                nc.vector.tensor_scalar(out=ish[:], in0=m.bitcast(I32),
                                        scalar1=1, scalar2=None,
                                        op0=mybir.AluOpType.logical_shift_right)
                y0i = pp_small.tile([P, 4, 2 * HL], I32, tag="y0i", name="y0i")
                nc.vector.tensor_scalar(out=y0i[:], in0=ish[:],
                                        scalar1=-1, scalar2=0x5F3759DF,
                                        op0=mybir.AluOpType.mult,
                                        op1=mybir.AluOpType.add)
                y = y0i.bitcast(F32)
                rstd = pp_small.tile([P, 4, 2 * HL], F32, tag="rstd", name="rstd")
                t_nr = pp_small.tile([P, 4, 2 * HL], F32, tag="t_nr", name="t_nr")
                cur = y
                for _it in range(2):
                    nc.vector.tensor_mul(t_nr[:], cur, cur)
                    nc.vector.tensor_mul(t_nr[:], t_nr[:], m[:])
                    nc.vector.tensor_scalar(out=t_nr[:], in0=t_nr[:], scalar1=-0.5,
                                            scalar2=1.5, op0=mybir.AluOpType.mult,
                                            op1=mybir.AluOpType.add)
                    nc.vector.tensor_mul(rstd[:], cur, t_nr[:])
                    cur = rstd[:]
                return xg, rstd

            def finish_tile(grp, ii, i):
                xg, rstd = grp

                def finish(xh, off, cos_sb, sin_sb, dstT):
                    # rope on raw x (rstd commutes with the rotation)
                    pre = pp.tile([P, HL, D], BF16, tag="pre", name="pre")
                    nc.vector.tensor_mul(pre[:], xh, bcast_heads(cos_sb[:, i]))
                    tmp = pp.tile([P, HL, RDIM], BF16, tag="tmp_rot", name="tmp")
                    nc.gpsimd.tensor_mul(
                        tmp[:, :, 0:16], xh[:, :, 16:32],
                        bcast_heads(sin_sb[:, i, 0:16]),
                    )
                    nc.gpsimd.tensor_mul(
                        tmp[:, :, 16:32], xh[:, :, 0:16],
                        bcast_heads(sin_sb[:, i, 16:32]),
                    )
                    nc.gpsimd.tensor_add(
                        pre[:, :, 0:RDIM], pre[:, :, 0:RDIM], tmp[:]
                    )
                    # apply rstd (per n,head scalar, broadcast along d)
                    a = pp.tile([P, HL, D], BF16, tag="a_norm", name="a")
                    rstd_b = bass.AP(
                        tensor=rstd.tensor,
                        offset=rstd.offset + ii * 2 * HL + off,
                        ap=[rstd.ap[0], [1, HL], [0, D]],
                    )
                    nc.vector.tensor_mul(a[:], pre[:], rstd_b)
                    # PE transpose per head pair, copy to [d, n] SBUF
                    for hp in range(3):
                        slot = next_slot()
                        nc.tensor.transpose(
                            slot, a[:, 2 * hp : 2 * hp + 2, :], ident[:]
                        )
                        nc.vector.tensor_copy(
                            dstT[:, hp, i * P : (i + 1) * P], slot
                        )

                finish(xg[:, 0, ii], 0, kcos_sb, ksin_sb, kT)
                finish(xg[:, 1, ii], HL, qcos_sb, qsin_sb, qT)

            # ---------------- attention emit helpers ----------------
            def emit_qk_exp(h, qc, filler):
                """Scores + exp + diag mask for (head, q-chunk). Returns pt list."""
                hp, hh = divmod(h, 2)
                hoff = 64 * hh
                nkt = 4 * (qc + 1)
                ngroups = nkt // 2
                pts = []
                for kg in range(ngroups):
                    ss = ps_s.tile([P, 2, QC], F32, tag="ps_s", name="ss")
                    for j in range(2):
                        kt = kg * 2 + j
                        doff = kt - 4 * qc
                        qstart = max(doff, 0) * P
                        nc.tensor.matmul(
                            ss[:, j, qstart:QC],
                            kT[hoff : hoff + 64, hp, kt * P : (kt + 1) * P],
                            qT[hoff : hoff + 64, hp, qc * QC + qstart : (qc + 1) * QC],
                            start=True, stop=True,
                        )
                    pt = pt_pool.tile([P, 2, QC], BF16, tag="pt", name="pt")
                    # exp only the causally-live q range (union over the pair)
                    qmin = max(kg * 2 - 4 * qc, 0) * P
                    nc.scalar.activation(
                        out=pt[:, :, qmin:], in_=ss[:, :, qmin:],
                        func=mybir.ActivationFunctionType.Exp,
                    )
                    for j in range(2):
                        doff = kg * 2 + j - 4 * qc
                        if doff >= 0:
                            nc.vector.tensor_mul(
                                pt[:, j, doff * P : (doff + 1) * P],
                                pt[:, j, doff * P : (doff + 1) * P],
                                tri_sb[:],
                            )
                    pts.append(pt)
                    # interleave prev head's AV units as PE filler
                    if filler:
                        n = -(-len(filler) // (ngroups - kg))
                        for _ in range(min(n, len(filler))):
                            filler.pop(0)()
                return pts

            def make_av_units(h, qc, pts, ypre3):
                """Per-kt AV emission closures + final normalize closure.

                The caller interleaves these between the next head's QK
                groups so the PE always has ready-to-run filler work while
                exp catches up (keeps the p-state ramped).
                """
                hp, hh = divmod(h, 2)
                nkt = 4 * (qc + 1)
                ys = ps_y.tile([P, 4, D + 1], F32, tag="ps_y", name="ys")

                def mk(kt):
                    def unit():
                        pt = pts[kt // 2]
                        j = kt % 2
                        for qs in range(4):
                            first = kt == 0 and qs == 0
                            last = kt == nkt - 1 and qs == 3
                            if (not first and not last
                                    and kt * P >= qc * QC + (qs + 1) * P):
                                continue
                            nc.tensor.matmul(
                                ys[:, qs, :],
                                pt[:, j, qs * P : (qs + 1) * P],
                                vall[:, kt, h, :],
                                start=first, stop=last,
                            )
                    return unit

                def fin():
                    r = recip_pool.tile([P, 4], F32, tag="recip", name="r")
                    nc.vector.reciprocal(out=r[:], in_=ys[:, :, D])
                    r_b = bass.AP(tensor=r.tensor, offset=r.offset,
                                  ap=[r.ap[0], r.ap[1], [0, D]])
                    nc.vector.tensor_mul(
                        ypre3[hp][:, :, hh, :], ys[:, :, 0:D], r_b)

                return [mk(kt) for kt in range(nkt)] + [fin]

            def emit_proj_nt(qc, ypre3, qs):
                """y transpose + projection + output store for one row tile."""
                nt_i = qc * 4 + qs
                yt = yt_pool.tile([P, 3, P], BF16, tag="yt", name="yt")
                for hp in range(3):
                    slot = next_slot()
                    nc.tensor.transpose(
                        slot, ypre3[hp][:, qs, :, :], ident[:]
                    )
                    nc.vector.tensor_copy(yt[:, hp, :], slot)
                osb = outsb_pool.tile([P, E], F32, tag="osb")
                po = ps_o.tile([P, 2, 256], F32, tag="ps_o", name="po")
                for oh in range(3):
                    sl = oh % 2
                    for ec in range(3):
                        nc.tensor.matmul(
                            po[:, sl, :],
                            yt[:, ec, :],
                            wt_sb[:, ec, oh * 256 : (oh + 1) * 256],
                            start=(ec == 0), stop=(ec == 2),
                        )
                    nc.vector.tensor_copy(
                        osb[:, oh * 256 : (oh + 1) * 256], po[:, sl, :]
                    )
                nc.sync.dma_start(out=out_t[nt_i], in_=osb[:])

            # ---------------- main pipelined schedule ----------------
            pending_av = []     # at most one (h, qc, pts, ypre3)
            pending_proj = []   # (qc, ypre3, qs) row-tile units
            ypre3_cur = None
            grp = preprocess_group(0, sq_eng=nc.vector)
            for ii in range(4):
                finish_tile(grp, ii, ii)
            for qc in range(NQC):
                ypre3_cur = [
                    ypre_pool.tile([P, 4, 2, D], BF16, tag=f"ypre{_i}",
                                   name=f"ypre{_i}")
                    for _i in range(3)
                ]
                for h in range(HL):
                    if qc == 0 and h == 1:
                        # bulk v/wt after both startup q/k groups' dispatches:
                        # the 3.8MB never queues ahead of the latency-critical
                        # tile loads on the shared DMA engines
                        nc.sync.dma_start(out=vall[:], in_=v_ext[:])
                        nc.sync.dma_start(out=wt_sb[:], in_=wt_ext[:])
                    filler = []
                    meta = None
                    if pending_av:
                        ph, pqc, ppts, pypre = pending_av.pop()
                        filler = make_av_units(ph, pqc, ppts, pypre)
                        meta = (ph, pqc, pypre)
                    pts = emit_qk_exp(h, qc, filler)
                    for u in filler:
                        u()
                    if meta is not None:
                        ph, pqc, pypre = meta
                        if ph == HL - 1:
                            pending_proj.extend(
                                (pqc, pypre, qs) for qs in range(4))
                        elif pending_proj:
                            emit_proj_nt(*pending_proj.pop(0))
                    pending_av.append((h, qc, pts, ypre3_cur))
                    # preprocess next q-chunk's tiles inside the head loop so
                    # their PE transposes never head an in-order stall at the
                    # chunk boundary
                    if qc < NQC - 1:
                        if h == 1:
                            grp = preprocess_group(4 * (qc + 1))
                        elif 2 <= h:
                            finish_tile(grp, h - 2, 4 * (qc + 1) + (h - 2))
            # drain
            ph, pqc, ppts, pypre = pending_av.pop()
            for u in make_av_units(ph, pqc, ppts, pypre):
                u()
            while pending_proj:
                emit_proj_nt(*pending_proj.pop(0))
            for qs in range(4):
                emit_proj_nt(pqc, pypre, qs)
    return nc


def _get_graph():
    if "nc" not in _CACHE:
        _CACHE["nc"] = build_graph()
    return _CACHE["nc"]


def _host_inputs(q, k, v, q_scale, k_scale, proj_w):
    perm = _head_perm()
    bf = ml_dtypes.bfloat16
    qcos, qsin = _build_tables(q_scale, invert_xpos=False)
    kcos, ksin = _build_tables(k_scale, invert_xpos=True)

    def tab_layout(t):
        # [N, w] -> [128, NT*w] matching sbuf tile [P, NT, w]
        w = t.shape[1]
        return np.ascontiguousarray(
            t.reshape(NT, P, w).transpose(1, 0, 2).reshape(P, NT * w)).astype(bf)

    qcos_r, qsin_r = tab_layout(qcos), tab_layout(qsin)
    kcos_r, ksin_r = tab_layout(kcos), tab_layout(ksin)
    # combined [P, NT, 2, D+RDIM]: slot 0 = k tables, slot 1 = q tables
    tabs = np.empty((P, NT, 2, D + RDIM), dtype=kcos_r.dtype)
    tabs[:, :, 0, 0:D] = kcos_r.reshape(P, NT, D)
    tabs[:, :, 0, D:] = ksin_r.reshape(P, NT, RDIM)
    tabs[:, :, 1, 0:D] = qcos_r.reshape(P, NT, D)
    tabs[:, :, 1, D:] = qsin_r.reshape(P, NT, RDIM)
    tabs_r = np.ascontiguousarray(tabs.reshape(P, NT * 2 * (D + RDIM)))
    # triangular mask for the diagonal [128,128] sub-block: tri[k, q] = q >= k
    tri = (np.arange(P)[None, :] >= np.arange(P)[:, None]).astype(np.float32)
    tri_r = np.ascontiguousarray(tri).astype(bf)

    in_maps = []
    for c in range(8):
        b = c // 2
        h0 = HL * (c % 2)
        cols = np.concatenate([(h0 + h) * D + perm for h in range(HL)])
        vcols = np.arange(h0 * D, (h0 + HL) * D)
        v_aug = np.ones((N, HL, D + 1), np.float32)
        v_aug[:, :, :D] = v[b][:, vcols].reshape(N, HL, D)
        wt_l = np.ascontiguousarray(proj_w[:, vcols].T)   # [384, 768]
        wt_r = np.ascontiguousarray(
            wt_l.reshape(3, P, E).transpose(1, 0, 2).reshape(P, 3 * E))
        in_maps.append({
            "q": np.ascontiguousarray(q[b][:, cols]).astype(bf),
            "k": np.ascontiguousarray(k[b][:, cols]).astype(bf),
            "v": np.ascontiguousarray(
                v_aug.reshape(NT, P, HL * (D + 1)).transpose(1, 0, 2)
                .reshape(P, NT * HL * (D + 1))).astype(bf),
            "wt": wt_r.astype(bf),
            "tabs": tabs_r,
            "tri": tri_r,
        })
    return in_maps


def kernel(q, k, v, q_scale, k_scale, proj_w, proj_b):
    nc = _get_graph()
    in_maps = _host_inputs(q, k, v, q_scale, k_scale, proj_w)
    res = run_bass_kernel_spmd(nc, in_maps, list(range(8)))
    out = np.empty((B, N, E), np.float32)
    for b in range(B):
        out[b] = res.results[2 * b]["out"] + res.results[2 * b + 1]["out"]
    out += proj_b[None, None, :].astype(np.float32)
    return out


# revision 35
# speedup vs baseline: 1.6040x; 1.0487x over previous
"""Distributed Trainium2 kernel for nn_Attention_64742337020012.

B=4, N=2048, E=768, H=12, D=64 causal attention with per-head RMS norm,
interleaved xpos RoPE, and output projection.

Sharding: 8 cores, core c owns batch c//2 and heads 6*(c%2) .. 6*(c%2)+6
(head-independent attention).  Each core computes full causal attention for
its 6 heads over all 2048 positions plus the partial output projection using
its 384 rows of proj_w^T; the host sums the two partial projections per batch
and adds the bias.

Pipeline (all matmuls bf16, f32 accumulation):
  1. q,k loaded bf16 (cast on host) in batched 4-tile group DMAs, roped on
     DVE via host-precomputed coefficient tables (head dim pre-permuted
     evens-first so the pair swap is two contiguous strided multiply-adds);
     rsqrt(ms) via Quake seed + 2 Newton steps on DVE, batched [P, 48]
     across the 4-tile group (k-side folds 1/sqrt(D):
     rsqrt(ssum+64eps) = 0.125*rstd, so exp needs no extra scale).
  2. q',k' transposed to [d, n] via PE transposes (identity moving matrix)
     into 8 rotating sub-bank PSUM slots, copied to SBUF on DVE.  No DMA
     transposes anywhere.
  3. Scores computed transposed (S^T[k,q] tiles), causal-trimmed moving
     range on diagonal chunks, exp on ACT straight out of PSUM over the
     causally-live column range, triangular mask multiply only on the
     [128,128] diagonal sub-block (DVE).
  4. AV with P^T stationary and [V | 1] moving gives y and the softmax
     denominator in one accumulation; per-row reciprocal normalizes.
     The PE stream is software-pipelined one head deep, with the previous
     head's AV matmuls interleaved between score-matmul groups as filler
     so the PE keeps busy (and its p-state up) while ACT's exp catches up.
  5. y transposed on PE, projected against the core's slice of proj_w^T in
     3x256-col chunks (projection row-tiles spread across head slots so
     PSUM bank recycling hides behind attention work); f32 partial written
     straight from SBUF.
Startup is latency-tuned: one merged rope-table DMA, q/k group loads ahead
of the bulk v/wt loads on the sync queue, and the next q-chunk's
preprocessing emitted inside the current head loop so its PE transposes
never head an in-order stall.
"""

import sys

sys.path.insert(0, "/opt/trn_rl_repo")

import numpy as np
import ml_dtypes

import concourse.bass as bass
import concourse.mybir as mybir
import concourse.tile as tile
from concourse.bass_utils import run_bass_kernel_spmd

# ----------------------------------------------------------------------------
# Workaround for this container's walrus build: the TileContext tail drain
# carries one SyncWait per outstanding semaphore, but CoreV3 CTRL codegen
# accepts only a single sync wait per instruction.  Split the waits across
# single-wait NOPs emitted right after the drain.
from concourse.vector_clock import ScopedClock as _ScopedClock


def _split_sync_waits(nc, inst, max_waits=1):
    si = inst.ins.sync_info
    if si is None:
        return
    waits = list(si.on_wait)
    if len(waits) <= max_waits:
        return
    inst.ins.sync_info = mybir.SyncInfo(
        on_wait=waits[:max_waits], on_update=list(si.on_update)
    )
    for i in range(max_waits, len(waits), max_waits):
        nop = nc.sync.nop(nofuse=True, hint="drain_wait_split")
        nop.ins.sync_info = mybir.SyncInfo(
            on_wait=waits[i : i + max_waits], on_update=[]
        )


def _patched_drain_and_barrier(self, tick_clock, wait_clock):
    nc = self.nc
    drain_inst = nc.sync.drain()
    wait_clock.add_sem_waits(
        drain_inst.ins, _ScopedClock({None: tick_clock.global_clock})
    )
    _split_sync_waits(nc, drain_inst)
    nc.all_engine_barrier()
    assert self.sems is not None
    popped = nc._tile_sem_poison_stack.pop()
    assert popped is self._sem_poison
    nc.clear_and_free_semaphores(list(self.sems.allocated().values()))
    nc.all_engine_barrier()


tile.TileContext._drain_and_barrier = _patched_drain_and_barrier


# Same walrus limitation, applied globally: any instruction carrying more
# than one SyncWait gets the extra waits hoisted onto same-engine NoOps
# inserted immediately before it in the BIR json (equivalent semantics: the
# engine's sequencer performs the waits in sequence).
import json as _json
import concourse.bass2jax as _bass2jax

_orig_compile_bir_kernel = _bass2jax.compile_bir_kernel


def _split_waits_in_bir(bir_json: bytes) -> bytes:
    j = _json.loads(bir_json)
    n_new = [0]
    for fn in j["functions"]:
        for bb in fn["blocks"]:
            insts = bb["instructions"]
            out = []
            for inst in insts:
                si = inst.get("sync_info")
                waits = (si or {}).get("on_wait") or []
                if len(waits) > 1:
                    for w in waits[:-1]:
                        n_new[0] += 1
                        out.append({
                            "engine": inst["engine"],
                            "ins": [], "outs": [],
                            "name": f"{inst['name']}-ws{n_new[0]}",
                            "opcode": "NoOp",
                            "sync_info": {"on_wait": [w], "on_update": []},
                        })
                    si["on_wait"] = [waits[-1]]
                out.append(inst)
            bb["instructions"] = out
    return _json.dumps(j).encode()


def _patched_compile_bir_kernel(bir_json, tmpdir, neff_name="file.neff"):
    return _orig_compile_bir_kernel(_split_waits_in_bir(bir_json), tmpdir, neff_name)


_bass2jax.compile_bir_kernel = _patched_compile_bir_kernel
# ----------------------------------------------------------------------------

B, N, E, H = 4, 2048, 768, 12
D = 64
RDIM = 32
EPS = 1e-6
XPOS_SCALE_BASE = 512.0
THETA = 10000.0

HL = 6            # heads per core
EL = HL * D       # 384 local embed cols
P = 128
NT = N // P       # 16 row tiles
QC = 512          # q chunk (columns of S^T tiles)
NQC = N // QC     # 4
F32 = mybir.dt.float32
BF16 = mybir.dt.bfloat16
I32 = mybir.dt.int32

_CACHE = {}


def _head_perm():
    """Per-head column permutation: rotary evens, rotary odds, passthrough."""
    p = list(range(0, RDIM, 2)) + list(range(1, RDIM, 2)) + list(range(RDIM, D))
    return np.array(p, dtype=np.int64)


def _build_tables(scale_vec, invert_xpos):
    """cosPt [N, 64], sinPt [N, 32] coefficient tables in permuted layout.

    slot i (i<16):    out = x_e[i]*cosPt[i] + x_o[i]*sinPt[i]
    slot 16+i:        out = x_o[i]*cosPt[16+i] + x_e[i]*sinPt[16+i]
    slot 32+j:        out = x[32+j]*cosPt[32+j]
    scale_vec: rms scale (q_scale/k_scale), indexed in ORIGINAL layout.
    Returns (cosPt, sinPt) including the xpos scale (inverted for k).
    """
    inv_freq = 1.0 / (THETA ** (np.arange(0, RDIM, 2, dtype=np.float64) / RDIM))
    t = np.arange(N, dtype=np.float64)
    freqs = t[:, None] * inv_freq[None, :]           # [N, 16]
    cos0, sin0 = np.cos(freqs), np.sin(freqs)
    base = (np.arange(0, RDIM, 2, dtype=np.float64) + 0.4 * RDIM) / (1.4 * RDIM)
    power = (t - N // 2) / XPOS_SCALE_BASE
    xsc = base[None, :] ** power[:, None]            # [N, 16]
    if invert_xpos:
        xsc = 1.0 / xsc
    sc = np.asarray(scale_vec, dtype=np.float64)
    cosPt = np.empty((N, D), dtype=np.float64)
    sinPt = np.empty((N, RDIM), dtype=np.float64)
    i = np.arange(16)
    cosPt[:, 0:16] = cos0 * xsc * sc[2 * i][None, :]
    cosPt[:, 16:32] = cos0 * xsc * sc[2 * i + 1][None, :]
    cosPt[:, 32:] = sc[RDIM:][None, :]
    sinPt[:, 0:16] = -sin0 * xsc * sc[2 * i + 1][None, :]
    sinPt[:, 16:32] = sin0 * xsc * sc[2 * i][None, :]
    return cosPt.astype(np.float32), sinPt.astype(np.float32)


def build_graph():
    nc = bass.Bass()
    q_ext = nc.declare_dram_parameter("q", [N, EL], BF16, isOutput=False)
    k_ext = nc.declare_dram_parameter("k", [N, EL], BF16, isOutput=False)
    v_ext = nc.declare_dram_parameter("v", [P, NT * HL * (D + 1)], BF16, isOutput=False)
    wt_ext = nc.declare_dram_parameter("wt", [P, 3 * E], BF16, isOutput=False)
    tabs_ext = nc.declare_dram_parameter(
        "tabs", [P, NT * 2 * (D + RDIM)], BF16, isOutput=False)
    tri_ext = nc.declare_dram_parameter("tri", [P, P], BF16, isOutput=False)
    out_ext = nc.declare_dram_parameter("out", [N, E], F32, isOutput=True)

    q_t4 = q_ext.rearrange("(g t p) e -> g p t e", t=4, p=P)
    k_t4 = k_ext.rearrange("(g t p) e -> g p t e", t=4, p=P)
    out_t = out_ext.rearrange("(t p) e -> t p e", p=P)

    def bcast_heads(ap, nh=HL):
        return bass.AP(tensor=ap.tensor, offset=ap.offset,
                       ap=[ap.ap[0], [0, nh], ap.ap[1]])

    with tile.TileContext(nc) as tc:
        with (
            tc.tile_pool(name="persist", bufs=1) as persist,
            tc.tile_pool(name="qk_in", bufs=10) as qk_in,
            tc.tile_pool(name="pp", bufs=8) as pp,
            tc.tile_pool(name="pp_small", bufs=4) as pp_small,
            tc.tile_pool(name="pt_pool", bufs=20) as pt_pool,
            tc.tile_pool(name="ypre", bufs=3) as ypre_pool,
            tc.tile_pool(name="yt_pool", bufs=3) as yt_pool,
            tc.tile_pool(name="recip", bufs=8) as recip_pool,
            tc.tile_pool(name="outsb", bufs=4) as outsb_pool,
            tc.tile_pool(name="ps_s", bufs=2, space="PSUM") as ps_s,
            tc.tile_pool(name="ps_y", bufs=2, space="PSUM") as ps_y,
            tc.tile_pool(name="ps_t", bufs=1, space="PSUM") as ps_t,
            tc.tile_pool(name="ps_o", bufs=1, space="PSUM") as ps_o,
        ):
            # ---------------- constants (host-prepared layouts) ----------------
            ident = persist.tile([P, P], BF16)
            from concourse.masks import make_identity
            make_identity(nc, ident)
            tabs_sb = persist.tile([P, NT, 2, D + RDIM], BF16)
            tri_sb = persist.tile([P, P], BF16)
            wt_sb = persist.tile([P, 3, E], BF16)
            vall = persist.tile([P, NT, HL, D + 1], BF16)
            # rope tables + tri on the sync queue (critical path: first
            # preprocess / first diag mask); bulk v/wt go on sync AFTER the
            # startup q/k tile loads (below).
            nc.sync.dma_start(out=tabs_sb[:], in_=tabs_ext[:])
            nc.sync.dma_start(out=tri_sb[:], in_=tri_ext[:])
            kcos_sb = tabs_sb[:, :, 0, 0:D]
            ksin_sb = tabs_sb[:, :, 0, D:]
            qcos_sb = tabs_sb[:, :, 1, 0:D]
            qsin_sb = tabs_sb[:, :, 1, D:]


            # transposed q', k': [128 = 2-head d, hp, n]
            qT = persist.tile([P, 3, N], BF16, name="qT")
            kT = persist.tile([P, 3, N], BF16, name="kT")

            # rotating PSUM transpose slots: 8 x [P, 128] bf16 in one bank
            psT = ps_t.tile([P, 8, P], BF16, name="psT")
            slot_ctr = [0]

            def next_slot():
                s = slot_ctr[0] % 8
                slot_ctr[0] += 1
                return psT[:, s, :]

            # ---------------- preprocess: 4-tile groups ----------------
            # The Newton-rsqrt chain runs once per GROUP of 4 tiles on
            # [P, 48] batches (12 DVE ops instead of 48), cutting both DVE
            # load and the startup latency to the first transposed q/k.
            def preprocess_group(i0, sq_eng=None):
                g = i0 // 4
                sq_eng = sq_eng or nc.gpsimd
                ssum4 = pp_small.tile([P, 4, 2 * HL], F32, tag="ssum4",
                                      name="ssum4")
                xg = qk_in.tile([P, 2, 4, HL, D], BF16, tag="xqk", name="x")
                nc.sync.dma_start(out=xg[:, 0], in_=k_t4[g])
                nc.sync.dma_start(out=xg[:, 1], in_=q_t4[g])
                for ii in range(4):
                    sq = pp.tile([P, 2, HL, D], BF16, tag="sq", name="sq")
                    sq_eng.tensor_mul(sq[:], xg[:, :, ii], xg[:, :, ii])
                    nc.vector.reduce_sum(ssum4[:, ii, :], sq[:],
                                         axis=mybir.AxisListType.X)
                # rstd via Quake seed + 2 Newton steps (DVE, batched x4).
                # k half: rsqrt(ssum + 64*eps) = 0.125 * rstd_k (folds 1/sqrt(D))
                # q half: rsqrt(ssum/64 + eps) = rstd_q
                m = pp_small.tile([P, 4, 2 * HL], F32, tag="m_ms", name="m")
                nc.vector.tensor_scalar(out=m[:, :, 0:HL],
                                        in0=ssum4[:, :, 0:HL],
                                        scalar1=float(D) * EPS, scalar2=None,
                                        op0=mybir.AluOpType.add)
                nc.vector.tensor_scalar(out=m[:, :, HL:],
                                        in0=ssum4[:, :, HL:],
                                        scalar1=1.0 / D, scalar2=EPS,
                                        op0=mybir.AluOpType.mult,
                                        op1=mybir.AluOpType.add)
                ish = pp_small.tile([P, 4, 2 * HL], I32, tag="ish", name="ish")
                nc.vector.tensor_scalar(out=ish[:], in0=m.bitcast(I32),
                                        scalar1=1, scalar2=None,
                                        op0=mybir.AluOpType.logical_shift_right)
                y0i = pp_small.tile([P, 4, 2 * HL], I32, tag="y0i", name="y0i")
                nc.vector.tensor_scalar(out=y0i[:], in0=ish[:],
                                        scalar1=-1, scalar2=0x5F3759DF,
                                        op0=mybir.AluOpType.mult,
                                        op1=mybir.AluOpType.add)
                y = y0i.bitcast(F32)
                rstd = pp_small.tile([P, 4, 2 * HL], F32, tag="rstd", name="rstd")
                t_nr = pp_small.tile([P, 4, 2 * HL], F32, tag="t_nr", name="t_nr")
                cur = y
                for _it in range(2):
                    nc.vector.tensor_mul(t_nr[:], cur, cur)
                    nc.vector.tensor_mul(t_nr[:], t_nr[:], m[:])
                    nc.vector.tensor_scalar(out=t_nr[:], in0=t_nr[:], scalar1=-0.5,
                                            scalar2=1.5, op0=mybir.AluOpType.mult,
                                            op1=mybir.AluOpType.add)
                    nc.vector.tensor_mul(rstd[:], cur, t_nr[:])
                    cur = rstd[:]
                return xg, rstd

            def finish_tile(grp, ii, i):
                xg, rstd = grp

                def finish(xh, off, cos_sb, sin_sb, dstT):
                    # rope on raw x (rstd commutes with the rotation)
                    pre = pp.tile([P, HL, D], BF16, tag="pre", name="pre")
                    nc.vector.tensor_mul(pre[:], xh, bcast_heads(cos_sb[:, i]))
                    tmp = pp.tile([P, HL, RDIM], BF16, tag="tmp_rot", name="tmp")
                    nc.gpsimd.tensor_mul(
                        tmp[:, :, 0:16], xh[:, :, 16:32],
                        bcast_heads(sin_sb[:, i, 0:16]),
                    )
                    nc.gpsimd.tensor_mul(
                        tmp[:, :, 16:32], xh[:, :, 0:16],
                        bcast_heads(sin_sb[:, i, 16:32]),
                    )
                    nc.gpsimd.tensor_add(
                        pre[:, :, 0:RDIM], pre[:, :, 0:RDIM], tmp[:]
                    )
                    # apply rstd (per n,head scalar, broadcast along d)
                    a = pp.tile([P, HL, D], BF16, tag="a_norm", name="a")
                    rstd_b = bass.AP(
                        tensor=rstd.tensor,
                        offset=rstd.offset + ii * 2 * HL + off,
                        ap=[rstd.ap[0], [1, HL], [0, D]],
                    )
                    nc.vector.tensor_mul(a[:], pre[:], rstd_b)
                    # PE transpose per head pair, copy to [d, n] SBUF
                    for hp in range(3):
                        slot = next_slot()
                        nc.tensor.transpose(
                            slot, a[:, 2 * hp : 2 * hp + 2, :], ident[:]
                        )
                        nc.vector.tensor_copy(
                            dstT[:, hp, i * P : (i + 1) * P], slot
                        )

                finish(xg[:, 0, ii], 0, kcos_sb, ksin_sb, kT)
                finish(xg[:, 1, ii], HL, qcos_sb, qsin_sb, qT)

            # ---------------- attention emit helpers ----------------
            def emit_qk_exp(h, qc, filler):
                """Scores + exp + diag mask for (head, q-chunk). Returns pt list."""
                hp, hh = divmod(h, 2)
                hoff = 64 * hh
                nkt = 4 * (qc + 1)
                ngroups = nkt // 2
                pts = []
                for kg in range(ngroups):
                    ss = ps_s.tile([P, 2, QC], F32, tag="ps_s", name="ss")
                    for j in range(2):
                        kt = kg * 2 + j
                        doff = kt - 4 * qc
                        qstart = max(doff, 0) * P
                        nc.tensor.matmul(
                            ss[:, j, qstart:QC],
                            kT[hoff : hoff + 64, hp, kt * P : (kt + 1) * P],
                            qT[hoff : hoff + 64, hp, qc * QC + qstart : (qc + 1) * QC],
                            start=True, stop=True,
                        )
                    pt = pt_pool.tile([P, 2, QC], BF16, tag="pt", name="pt")
                    # exp only the causally-live q range (union over the pair)
                    qmin = max(kg * 2 - 4 * qc, 0) * P
                    nc.scalar.activation(
                        out=pt[:, :, qmin:], in_=ss[:, :, qmin:],
                        func=mybir.ActivationFunctionType.Exp,
                    )
                    for j in range(2):
                        doff = kg * 2 + j - 4 * qc
                        if doff >= 0:
                            nc.vector.tensor_mul(
                                pt[:, j, doff * P : (doff + 1) * P],
                                pt[:, j, doff * P : (doff + 1) * P],
                                tri_sb[:],
                            )
                    pts.append(pt)
                    # interleave prev head's AV units as PE filler
                    if filler:
                        n = -(-len(filler) // (ngroups - kg))
                        for _ in range(min(n, len(filler))):
                            filler.pop(0)()
                return pts

            def make_av_units(h, qc, pts, ypre3):
                """Per-kt AV emission closures + final normalize closure.

                The caller interleaves these between the next head's QK
                groups so the PE always has ready-to-run filler work while
                exp catches up (keeps the p-state ramped).
                """
                hp, hh = divmod(h, 2)
                nkt = 4 * (qc + 1)
                ys = ps_y.tile([P, 4, D + 1], F32, tag="ps_y", name="ys")

                def mk(kt):
                    def unit():
                        pt = pts[kt // 2]
                        j = kt % 2
                        for qs in range(4):
                            first = kt == 0 and qs == 0
                            last = kt == nkt - 1 and qs == 3
                            if (not first and not last
                                    and kt * P >= qc * QC + (qs + 1) * P):
                                continue
                            nc.tensor.matmul(
                                ys[:, qs, :],
                                pt[:, j, qs * P : (qs + 1) * P],
                                vall[:, kt, h, :],
                                start=first, stop=last,
                            )
                    return unit

                def fin():
                    r = recip_pool.tile([P, 4], F32, tag="recip", name="r")
                    nc.vector.reciprocal(out=r[:], in_=ys[:, :, D])
                    r_b = bass.AP(tensor=r.tensor, offset=r.offset,
                                  ap=[r.ap[0], r.ap[1], [0, D]])
                    nc.vector.tensor_mul(
                        ypre3[hp][:, :, hh, :], ys[:, :, 0:D], r_b)

                return [mk(kt) for kt in range(nkt)] + [fin]

            def emit_proj_nt(qc, ypre3, qs):
                """y transpose + projection + output store for one row tile."""
                nt_i = qc * 4 + qs
                yt = yt_pool.tile([P, 3, P], BF16, tag="yt", name="yt")
                for hp in range(3):
                    slot = next_slot()
                    nc.tensor.transpose(
                        slot, ypre3[hp][:, qs, :, :], ident[:]
                    )
                    nc.vector.tensor_copy(yt[:, hp, :], slot)
                osb = outsb_pool.tile([P, E], F32, tag="osb")
                po = ps_o.tile([P, 2, 256], F32, tag="ps_o", name="po")
                for oh in range(3):
                    sl = oh % 2
                    for ec in range(3):
                        nc.tensor.matmul(
                            po[:, sl, :],
                            yt[:, ec, :],
                            wt_sb[:, ec, oh * 256 : (oh + 1) * 256],
                            start=(ec == 0), stop=(ec == 2),
                        )
                    nc.vector.tensor_copy(
                        osb[:, oh * 256 : (oh + 1) * 256], po[:, sl, :]
                    )
                nc.sync.dma_start(out=out_t[nt_i], in_=osb[:])

            # ---------------- main pipelined schedule ----------------
            pending_av = []     # at most one (h, qc, pts, ypre3)
            pending_proj = []   # (qc, ypre3, qs) row-tile units
            ypre3_cur = None
            grp = preprocess_group(0, sq_eng=nc.vector)
            for ii in range(4):
                finish_tile(grp, ii, ii)
            for qc in range(NQC):
                ypre3_cur = [
                    ypre_pool.tile([P, 4, 2, D], BF16, tag=f"ypre{_i}",
                                   name=f"ypre{_i}")
                    for _i in range(3)
                ]
                for h in range(HL):
                    if qc == 0 and h == 1:
                        # bulk v/wt after both startup q/k groups' dispatches:
                        # the 3.8MB never queues ahead of the latency-critical
                        # tile loads on the shared DMA engines
                        nc.sync.dma_start(out=vall[:], in_=v_ext[:])
                        nc.sync.dma_start(out=wt_sb[:], in_=wt_ext[:])
                    filler = []
                    meta = None
                    if pending_av:
                        ph, pqc, ppts, pypre = pending_av.pop()
                        filler = make_av_units(ph, pqc, ppts, pypre)
                        meta = (ph, pqc, pypre)
                    pts = emit_qk_exp(h, qc, filler)
                    for u in filler:
                        u()
                    if meta is not None:
                        ph, pqc, pypre = meta
                        if ph == HL - 1:
                            pending_proj.extend(
                                (pqc, pypre, qs) for qs in range(4))
                        elif pending_proj:
                            emit_proj_nt(*pending_proj.pop(0))
                    pending_av.append((h, qc, pts, ypre3_cur))
                    # preprocess next q-chunk's tiles inside the head loop so
                    # their PE transposes never head an in-order stall at the
                    # chunk boundary
                    if qc < NQC - 1:
                        if h == 1:
                            grp = preprocess_group(4 * (qc + 1))
                        elif 2 <= h:
                            finish_tile(grp, h - 2, 4 * (qc + 1) + (h - 2))
            # drain
            ph, pqc, ppts, pypre = pending_av.pop()
            for u in make_av_units(ph, pqc, ppts, pypre):
                u()
            while pending_proj:
                emit_proj_nt(*pending_proj.pop(0))
            for qs in range(4):
                emit_proj_nt(pqc, pypre, qs)
    return nc


def _get_graph():
    if "nc" not in _CACHE:
        _CACHE["nc"] = build_graph()
    return _CACHE["nc"]


def _host_inputs(q, k, v, q_scale, k_scale, proj_w):
    perm = _head_perm()
    bf = ml_dtypes.bfloat16
    qcos, qsin = _build_tables(q_scale, invert_xpos=False)
    kcos, ksin = _build_tables(k_scale, invert_xpos=True)

    def tab_layout(t):
        # [N, w] -> [128, NT*w] matching sbuf tile [P, NT, w]
        w = t.shape[1]
        return np.ascontiguousarray(
            t.reshape(NT, P, w).transpose(1, 0, 2).reshape(P, NT * w)).astype(bf)

    qcos_r, qsin_r = tab_layout(qcos), tab_layout(qsin)
    kcos_r, ksin_r = tab_layout(kcos), tab_layout(ksin)
    # combined [P, NT, 2, D+RDIM]: slot 0 = k tables, slot 1 = q tables
    tabs = np.empty((P, NT, 2, D + RDIM), dtype=kcos_r.dtype)
    tabs[:, :, 0, 0:D] = kcos_r.reshape(P, NT, D)
    tabs[:, :, 0, D:] = ksin_r.reshape(P, NT, RDIM)
    tabs[:, :, 1, 0:D] = qcos_r.reshape(P, NT, D)
    tabs[:, :, 1, D:] = qsin_r.reshape(P, NT, RDIM)
    tabs_r = np.ascontiguousarray(tabs.reshape(P, NT * 2 * (D + RDIM)))
    # triangular mask for the diagonal [128,128] sub-block: tri[k, q] = q >= k
    tri = (np.arange(P)[None, :] >= np.arange(P)[:, None]).astype(np.float32)
    tri_r = np.ascontiguousarray(tri).astype(bf)

    in_maps = []
    for c in range(8):
        b = c // 2
        h0 = HL * (c % 2)
        cols = np.concatenate([(h0 + h) * D + perm for h in range(HL)])
        vcols = np.arange(h0 * D, (h0 + HL) * D)
        v_aug = np.ones((N, HL, D + 1), np.float32)
        v_aug[:, :, :D] = v[b][:, vcols].reshape(N, HL, D)
        wt_l = np.ascontiguousarray(proj_w[:, vcols].T)   # [384, 768]
        wt_r = np.ascontiguousarray(
            wt_l.reshape(3, P, E).transpose(1, 0, 2).reshape(P, 3 * E))
        in_maps.append({
            "q": np.ascontiguousarray(q[b][:, cols]).astype(bf),
            "k": np.ascontiguousarray(k[b][:, cols]).astype(bf),
            "v": np.ascontiguousarray(
                v_aug.reshape(NT, P, HL * (D + 1)).transpose(1, 0, 2)
                .reshape(P, NT * HL * (D + 1))).astype(bf),
            "wt": wt_r.astype(bf),
            "tabs": tabs_r,
            "tri": tri_r,
        })
    return in_maps


def kernel(q, k, v, q_scale, k_scale, proj_w, proj_b):
    nc = _get_graph()
    in_maps = _host_inputs(q, k, v, q_scale, k_scale, proj_w)
    res = run_bass_kernel_spmd(nc, in_maps, list(range(8)))
    out = np.empty((B, N, E), np.float32)
    for b in range(B):
        out[b] = res.results[2 * b]["out"] + res.results[2 * b + 1]["out"]
    out += proj_b[None, None, :].astype(np.float32)
    return out
